# revision 33
# baseline (speedup 1.0000x reference)
"""Trainium2 Bass kernel for nn_Attention_layer (GNN message passing attention).

Math (see harness reference):
  x_Q = [input_x, pe_Q]  (N, 1024);  x_K = [input_x, pe_K]
  Q = x_Q @ WQ[h] + qb;  K = x_K @ WK[h] + kb;  V = input_x @ WV[h] (+vb=0)
  attn = softmax(Q K^T / 16, axis=k);  out = concat_h(attn @ V) @ lin_w.T + lin_b

Distribution: 8 NeuronCores, query-dim (N) sharded 512 rows/core; K/V work
replicated (no collectives).  Per core, transposed domain (scores^T [k, q]).

Rev B design (head-group-major + PSUM PV accumulation + split exp engines):
  - iteration = (mg, kc): head-group mg in {0,1} (4 heads), k-chunk kc in
    0..31 (128 nodes). All 32 kc of mg=0 first, then mg=1.
  - scores: one 4-bank PSUM tensor st [128, 2048]; 4 matmuls (contraction
    hd=32, row-tiled via tile_position (32j, 0)).
  - exp split across engines by column (rows of a softmax stay on one
    engine so approximation error cancels in Z): ScalarE does true exp on
    cols [0, SPLIT); VectorE does a Schraudolph bit-trick exp on the rest
    (one tensor_scalar affine into int16 bits == bf16 exp, ~1.8% rms,
    zero-mean; confined to 2 of 8 heads -> ~8e-3 output rel err).
  - PV+Z fused: V augmented with a ones column per head (vt [.., 8, 33]);
    one M=33 matmul per head (tile cols 0 / 64) accumulates attn_x^T AND
    the softmax denominator Z directly in a persistent 2-bank PSUM tile
    across all 32 kc (no vector adds, no separate Z matmuls).
  - projections (Q/K/V on PE, bias-add/copy on VectorE; GpSimd cannot
    read PSUM) are interleaved
    just-in-time into the iteration stream, K proj for head-group mg
    scheduled inside mg's own half.
  - epilogue per mg (overlapped with the other half's stream): gather Z
    rows via selector matmul, fast reciprocal, broadcast via outer-product
    matmul, normalize; final linear uses a host-permuted lin_w so the
    PSUM partition layout feeds it directly; lin_w rows are zeroed for
    junk partitions.
"""

import os
import sys
import math
import numpy as np
import ml_dtypes

for _p in ("/opt/trn_rl_repo", "/root/.axon_site/_ro/trn_rl_repo"):
    if os.path.isdir(_p) and _p not in sys.path:
        sys.path.insert(0, _p)

N = 4096
IND = 256          # input_x dim
QKD = 1024         # concat dim for Q/K projections
H = 8              # heads
HD = 32            # head dim
HID = 256          # H * HD
NCORES = 8
NQ = N // NCORES   # 512 query rows per core
SCALE = 1.0 / 16.0  # 1/sqrt(HID)

# exp engine split: ScalarE (true exp) takes score cols [0, SPLIT) = heads
# j0..j2 (tensors stAB+stC); VectorE (Schraudolph) takes head j3 (stD).
SPLIT = 1536
EXP_A = 8.0 / math.log(2.0)        # 128/(16 ln2): bf16-bits slope on raw scores
EXP_B = 16256.0 - 7.4              # 127<<7 minus rms-centering constant

_CACHE = {}


def _build_nc():
    from contextlib import ExitStack
    import concourse.bacc as bacc
    import concourse.tile as tile
    import concourse.mybir as mybir
    from concourse.bass import ds, ts

    f32 = mybir.dt.float32
    bf16 = mybir.dt.bfloat16
    i16 = mybir.dt.int16
    Exp = mybir.ActivationFunctionType.Exp
    mult = mybir.AluOpType.mult
    add = mybir.AluOpType.add

    nc = bacc.Bacc("TRN2", target_bir_lowering=False, debug=False,
                   num_devices=NCORES)

    # Z gather / broadcast selector constants
    selz_np = np.zeros((128, 2), dtype=np.float16)
    selz_np[32, 0] = 1.0
    selz_np[96, 1] = 1.0
    bselm_np = np.zeros((2, 128), dtype=np.float16)
    bselm_np[0, 0:32] = 1.0
    bselm_np[1, 64:96] = 1.0

    # ---- DRAM I/O (per-core shards prepared on host) ----
    xkT = nc.dram_tensor("xkT", [QKD, N], bf16, kind="ExternalInput")   # [x;peK]^T
    xqT = nc.dram_tensor("xqT", [QKD, NQ], bf16, kind="ExternalInput")  # [x;peQ]^T rows blk
    wq = nc.dram_tensor("wq", [QKD, HID], bf16, kind="ExternalInput")   # [d,(h,hd)]
    wk = nc.dram_tensor("wk", [QKD, HID], bf16, kind="ExternalInput")
    wv = nc.dram_tensor("wv", [IND, HID], bf16, kind="ExternalInput")
    lwP = nc.dram_tensor("lwP", [4 * 128, HID], bf16, kind="ExternalInput")  # permuted lin_w.T
    bias4 = nc.dram_tensor("bias4", [128, 8], f32, kind="ExternalInput")  # [p, 4m+i]
    out = nc.dram_tensor("out", [HID, NQ], f32, kind="ExternalOutput")   # out^T


    with tile.TileContext(nc) as tc, ExitStack() as ctx:
        consts = ctx.enter_context(tc.tile_pool(name="consts", bufs=1))
        big = ctx.enter_context(tc.tile_pool(name="big", bufs=1))
        ptp = ctx.enter_context(tc.tile_pool(name="ptp", bufs=6))
        stp = ctx.enter_context(tc.tile_pool(name="stp", bufs=1, space="PSUM"))

        # ---- SBUF tiles ----
        xkt = big.tile([128, 8, N], bf16, tag="xkt")       # x_K^T  (8 c-chunks)
        xqt = big.tile([128, 8, NQ], bf16, tag="xqt")      # x_Q^T block
        wqt = consts.tile([128, 8, HID], bf16, tag="wqt")
        wkt = consts.tile([128, 8, HID], bf16, tag="wkt")
        wvt = consts.tile([128, 2, HID], bf16, tag="wvt")
        lwt = consts.tile([128, 4, HID], bf16, tag="lwt")  # permuted lin_w.T
        bt = consts.tile([128, 8], f32, tag="bt")          # [p, 4m+i]
        selz = consts.tile([128, 2], bf16, tag="selz")
        bselm = consts.tile([2, 128], bf16, tag="bselm")

        kt = big.tile([128, 2, N], bf16, tag="kt")         # K^T rows (h,hd)
        qt = big.tile([128, 2, NQ], bf16, tag="qt")        # Q^T
        vt = big.tile([128, 32, 8, 33], bf16, tag="vt")    # V node-major, +ones col
        pvs = big.tile([128, 2, 2, 512], f32, tag="pvs")   # PV+Z psum copies per mg
        attn2 = big.tile([128, 4, 512], bf16, tag="attn2")  # normalized attn_x^T
        zrm = big.tile([2, 2, 2, NQ], f32, tag="zrm")      # 1/Z [row, mg, b, q]
        zrh = big.tile([2, 2, 2, NQ], bf16, tag="zrh")     # bf16 1/Z for PE
        zsb = big.tile([128, 2, NQ], bf16, tag="zsb")      # bf16 copy of pvz
        outsb = big.tile([128, 2, NQ], f32, tag="outsb")

        # ---- persistent PSUM: separate score tensors per reader engine so
        # WAR chains stay independent (tile dep tracking is per-tensor) ----
        stAB = stp.tile([128, 2, NQ], f32, tag="stAB", name="stAB")  # exp1 (ACT)
        stC = stp.tile([128, NQ], f32, tag="stC", name="stC")        # exp2 (ACT)
        stD = stp.tile([128, NQ], f32, tag="stD", name="stD")        # schr (DVE)
        pvz = stp.tile([128, 2, NQ], f32, tag="pvz", name="pvz")     # 2 banks

        # ---- const / weight DMAs, ordered by first consumer ----
        xkT_r = xkT.rearrange("(c p) (n q) -> n p c q", p=128, q=512)
        xqT_r = xqT.rearrange("(c p) q -> p c q", p=128)
        wq_r = wq.rearrange("(c p) o -> p c o", p=128)
        nc.sync.dma_start(wqt[:, :4], wq_r[:, :4])
        nc.sync.dma_start(xqt[:, :4], xqT_r[:, :4])
        nc.sync.dma_start(wqt[:, 4:], wq_r[:, 4:])
        nc.sync.dma_start(wkt[:], wk.rearrange("(c p) o -> p c o", p=128))
        nc.sync.dma_start(xkt[:, :, ds(0, 128)], xkT_r[0][:, :, ds(0, 128)])
        nc.sync.dma_start(xqt[:, 4:], xqT_r[:, 4:])
        nc.sync.dma_start(bt[:], bias4[:])
        nc.sync.dma_start(wvt[:], wv.rearrange("(c p) o -> p c o", p=128))
        nc.sync.dma_start(xkt[:, :, ds(128, 384)], xkT_r[0][:, :, ds(128, 384)])
        for n in range(1, 8):
            nc.sync.dma_start(xkt[:, :, ts(n, 512)], xkT_r[n])
        nc.sync.dma_start(lwt[:], lwP.rearrange("(c p) o -> p c o", p=128))
        nc.sync.dma_start(selz[:], nc.inline_tensor(
            selz_np.astype(ml_dtypes.bfloat16), name="selz_c")[:])
        nc.sync.dma_start(bselm[:], nc.inline_tensor(
            bselm_np.astype(ml_dtypes.bfloat16), name="bselm_c")[:])

        # PE clock warmup: dependency-free dummy matmuls on a locally
        # memset tile start right after the preamble and ramp the PE p-state
        # while the input DMAs land.
        warm = big.tile([128, 512], bf16, tag="warm")
        nc.vector.memset(warm[:], 1.0)
        for w in range(8):
            nc.tensor.matmul(stAB[:, 0, :], warm[:, ds(0, 128)], warm[:],
                             start=True, stop=True)
        # preload the ACT exp table set while DMAs land
        actwarm = consts.tile([8, 16], f32, tag="actwarm")
        nc.vector.memset(actwarm[:], 0.0)
        nc.scalar.activation(actwarm[:], actwarm[:], Exp)
        # ones-column of augmented V (never overwritten: V copies skip col 32)
        for h in range(H):
            nc.vector.memset(vt[:, :, h, ds(32, 1)], 1.0)

        # PV+Z accumulates with start=False onto explicitly zeroed banks
        # (two start=True matmuls sharing a bank would re-zero each other)
        nc.vector.memset(pvz[:], 0.0)

        # ---- projection helpers (PE matmuls + GpSimd bias-add/copy) ----
        def q_proj_unit(m, lo, nmm):
            ps = stp.tile([128, NQ], f32, tag="pzv", bufs=1, name=f"qp{m}_{lo}")
            for c in range(lo, lo + nmm):
                nc.tensor.matmul(ps[:, :NQ], wqt[:, c, ts(m, 128)], xqt[:, c, :],
                                 start=(c == 0), stop=(c == 7))
            if lo + nmm == 8:
                nc.vector.tensor_scalar_add(qt[:, m, :], ps[:, :NQ],
                                            bt[:, 4 * m + 0:4 * m + 1])
            return ps

        qproj_open = {}

        def q_proj(m, half):
            if half == 0:
                qproj_open[m] = q_proj_unit(m, 0, 4)
            else:
                ps = qproj_open.pop(m)
                for c in range(4, 8):
                    nc.tensor.matmul(ps[:, :NQ], wqt[:, c, ts(m, 128)],
                                     xqt[:, c, :], start=False, stop=(c == 7))
                nc.vector.tensor_scalar_add(qt[:, m, :], ps[:, :NQ],
                                            bt[:, 4 * m + 0:4 * m + 1])

        def k_proj_narrow(n, m, lo, w):
            ps = stp.tile([128, NQ], f32, tag="pzk", bufs=1, name=f"kn{n}_{m}_{lo}")
            for c in range(8):
                nc.tensor.matmul(ps[:, :w], wkt[:, c, ts(m, 128)],
                                 xkt[:, c, ds(512 * n + lo, w)],
                                 start=(c == 0), stop=(c == 7))
            nc.vector.tensor_scalar_add(kt[:, m, ds(512 * n + lo, w)], ps[:, :w],
                                        bt[:, 4 * m + 1:4 * m + 2])

        kproj_open = {}

        def k_proj_quarter(n, m, qtr):
            if qtr == 0:
                ps = stp.tile([128, NQ], f32, tag="pzk", bufs=1, name=f"kp{n}_{m}")
                kproj_open[(n, m)] = ps
            else:
                ps = kproj_open[(n, m)]
            for c in range(2 * qtr, 2 * qtr + 2):
                nc.tensor.matmul(ps[:, :512], wkt[:, c, ts(m, 128)],
                                 xkt[:, c, ts(n, 512)],
                                 start=(c == 0), stop=(c == 7))
            if qtr == 3:
                del kproj_open[(n, m)]
                nc.vector.tensor_scalar_add(kt[:, m, ts(n, 512)], ps[:, :512],
                                            bt[:, 4 * m + 1:4 * m + 2])

        def v_proj_unit(kc):
            ps = stp.tile([128, NQ], f32, tag="pzv", bufs=1, name=f"vp{kc}")
            for c in range(2):
                nc.tensor.matmul(ps[:, :HID], xkt[:, c, ds(128 * kc, 128)],
                                 wvt[:, c, :], start=(c == 0), stop=(c == 1))
            # strided copy into augmented V layout (skips the ones column)
            nc.vector.tensor_copy(out=vt[:, kc, :, ds(0, 32)], in_=ps[:, :HID])

        # ---- epilogue per head-group ----
        def epilogue_zsb(b):
            nc.vector.tensor_copy(out=zsb[:, b, :], in_=pvz[:, b])

        def epilogue_copy(mg, b):
            nc.vector.tensor_copy(out=pvs[:, mg, b], in_=pvz[:, b])

        def pvz_clear():
            nc.vector.memset(pvz[:], 0.0)

        def epilogue_zq(mg, b):
            zq = stp.tile([128, NQ], f32, tag="pzv", bufs=1, name=f"zq{mg}_{b}")
            nc.tensor.matmul(zq[ds(0, 2), :NQ], selz[:], zsb[:, b, :],
                             start=True, stop=True)
            nc.vector.reciprocal_approx_fast(zrm[ds(0, 2), mg, b, :],
                                             zq[ds(0, 2), :NQ])
            nc.vector.tensor_copy(out=zrh[ds(0, 2), mg, b, :],
                                  in_=zrm[ds(0, 2), mg, b, :])

        def epilogue_norm(mg, b):
            psb = stp.tile([128, NQ], f32, tag="pzv", bufs=1, name=f"psb{mg}_{b}")
            nc.tensor.matmul(psb[:, :NQ], bselm[:], zrh[:, mg, b, :],
                             start=True, stop=True)
            nc.vector.tensor_tensor(attn2[:, 2 * mg + b, :], pvs[:, mg, b, :],
                                    psb[:, :NQ], mult)

        # ---- minimal prologue: what iteration 0 needs ----
        q_proj(0, 0)
        q_proj(0, 1)
        k_proj_narrow(0, 0, 0, 128)
        v_proj_unit(0)
        k_proj_narrow(0, 0, 128, 384)

        # ---- interleaved work schedule: slot i = iteration i ----
        pre_work = {}
        post_work = {}

        def at(i, fn):
            pre_work.setdefault(i, []).append(fn)

        def at_post(i, fn):
            post_work.setdefault(i, []).append(fn)

        # K proj m=0 tiles 1..7 during the preceding 4 iterations
        for n in range(1, 8):
            for qtr in range(4):
                at(max(1, 4 * (n - 1) + qtr),
                   lambda n=n, qtr=qtr: k_proj_quarter(n, 0, qtr))
        # K proj m=1 tile 0 late in the first half; tiles 1..7 in second half
        at(28, lambda: k_proj_narrow(0, 1, 0, 256))
        at(30, lambda: k_proj_narrow(0, 1, 256, 256))
        for n in range(1, 8):
            for qtr in range(4):
                at(32 + 4 * (n - 1) + qtr,
                   lambda n=n, qtr=qtr: k_proj_quarter(n, 1, qtr))
        # V chunks a couple of iterations ahead of their PV use
        for kc in range(1, 32):
            at(max(0, kc - 2), lambda kc=kc: v_proj_unit(kc))
        # Q proj m=1 before the second half
        at(22, lambda: q_proj(1, 0))
        at(26, lambda: q_proj(1, 1))
        # head-group 0 epilogue, overlapped with the mg=1 stream
        at_post(33, lambda: epilogue_zsb(0))
        at_post(33, lambda: epilogue_zsb(1))
        at_post(34, lambda: epilogue_copy(0, 0))
        at_post(34, lambda: epilogue_zq(0, 0))
        at_post(35, lambda: epilogue_copy(0, 1))
        at_post(35, lambda: epilogue_zq(0, 1))
        at_post(35, lambda: pvz_clear())
        at_post(36, lambda: epilogue_norm(0, 0))
        at_post(37, lambda: epilogue_norm(0, 1))

        # ---- PV+Z unit: matmuls accumulating in persistent PSUM ----
        def pvz_unit(ptA, ptB, mg, kc):
            for half in range(2):
                for b in range(2):
                    j = 2 * b + half
                    h = 4 * mg + j
                    rhs = (ptA[:, j, :] if j < 3 else ptB[:].bitcast(bf16))
                    nc.tensor.matmul(
                        pvz[ds(64 * half, 33), b, :],
                        vt[:, kc, h, :], rhs,
                        start=False, stop=(kc == 31),
                        tile_position=(0, 64 * half),
                        skip_group_check=True)

        # ---- main loop: 64 iterations of (mg, kc) ----
        pending = []
        for i in range(64):
            mg, kc = divmod(i, 32)[0], i % 32
            for fn in pre_work.get(i, []):
                fn()
            ptA = ptp.tile([128, 3, NQ], bf16, tag="ptA", name="ptA")
            ptB = ptp.tile([128, NQ], i16, tag="ptB", name="ptB")
            sdst = [stAB[:, 0, :], stAB[:, 1, :], stC[:, :], stD[:, :]]
            for j in range(4):
                nc.tensor.matmul(
                    sdst[j],
                    kt[ds(32 * j, 32), mg, ds(128 * kc, 128)],
                    qt[ds(32 * j, 32), mg, :],
                    start=True, stop=True,
                    tile_position=(32 * j, 0))
            nc.scalar.activation(ptA[:, 0:2, :], stAB[:], Exp, scale=SCALE)
            nc.scalar.activation(ptA[:, 2, :], stC[:], Exp, scale=SCALE)
            nc.vector.tensor_scalar(ptB[:], stD[:], EXP_A, EXP_B, mult, add)
            # hold mg=1's first PV units two extra slots so the mg=0
            # epilogue (which must finish reading/clearing pvz first) can
            # spread out; catch back up with double flushes.
            n_flush = {34: 0, 35: 0, 36: 2, 37: 2}.get(i, 1)
            for _ in range(n_flush):
                if len(pending) >= 2:
                    pvz_unit(*pending.pop(0))
            pending.append((ptA, ptB, mg, kc))
            for fn in post_work.get(i, []):
                fn()
        for args in pending:
            pvz_unit(*args)

        # ---- tail: head-group 1 epilogue + final linear (lin reuses the
        # dead score PSUM banks; per-bank chains interleave) ----
        def tdummy(n=1):
            for _ in range(n):
                nc.tensor.matmul(stAB[:, 0, :], warm[:, ds(0, 128)], warm[:],
                                 start=True, stop=True)

        epilogue_zsb(0)
        epilogue_zsb(1)
        # final linear: mg=0 chunks accumulate as soon as the score banks die
        for mo, ps in ((0, stC), (1, stD)):
            for c in range(2):
                nc.tensor.matmul(ps[:, :NQ], lwt[:, c, ts(mo, 128)],
                                 attn2[:, c, :], start=(c == 0), stop=False)
        epilogue_zq(1, 0)
        epilogue_copy(1, 0)
        epilogue_zq(1, 1)
        tdummy(2)
        epilogue_copy(1, 1)
        epilogue_norm(1, 0)
        for mo, ps in ((0, stC), (1, stD)):
            nc.tensor.matmul(ps[:, :NQ], lwt[:, 2, ts(mo, 128)],
                             attn2[:, 2, :], start=False, stop=False)
        epilogue_norm(1, 1)
        out_r = out.rearrange("(m p) q -> p m q", p=128)
        for mo, ps in ((0, stC), (1, stD)):
            nc.tensor.matmul(ps[:, :NQ], lwt[:, 3, ts(mo, 128)],
                             attn2[:, 3, :], start=False, stop=True)
            nc.vector.tensor_scalar_add(outsb[:, mo, :], ps[:, :NQ],
                                        bt[:, 4 * mo + 3:4 * mo + 4])
            nc.sync.dma_start(out_r[:, mo], outsb[:, mo, :])

    nc.compile()
    return nc


def _get_nc():
    if "nc" not in _CACHE:
        _CACHE["nc"] = _build_nc()
    return _CACHE["nc"]


def _prep_in_maps(input_x, pe_Q, pe_K, WQ, WK, WV, Q_bias, K_bias, V_bias,
                  lin_w, lin_b):
    bf = ml_dtypes.bfloat16
    x_kT = np.ascontiguousarray(
        np.concatenate([input_x, pe_K], axis=1).T.astype(bf))       # [1024, 4096]
    x_q = np.concatenate([input_x, pe_Q], axis=1)                   # [4096, 1024]
    wq2 = np.ascontiguousarray(
        WQ.transpose(1, 0, 2).reshape(QKD, HID).astype(bf))         # [d,(h,hd)]
    wk2 = np.ascontiguousarray(WK.transpose(1, 0, 2).reshape(QKD, HID).astype(bf))
    wv2 = np.ascontiguousarray(WV.transpose(1, 0, 2).reshape(IND, HID).astype(bf))
    # permuted lin_w.T for the PSUM partition layout: chunk c = 2*mg + b,
    # partition p<32 -> head (4mg+2b) row p; 64<=p<96 -> head (4mg+2b+1)
    # row p-64; other partitions (Z rows + junk) get zero weights.
    lwT = lin_w.T  # [HID_in (h, hd), HID_out]
    lwP = np.zeros((4 * 128, HID), np.float32)
    for c in range(4):
        mg, b = divmod(c, 2)
        h_lo = 4 * mg + 2 * b
        lwP[c * 128 + 0:c * 128 + 32] = lwT[32 * h_lo:32 * h_lo + 32]
        lwP[c * 128 + 64:c * 128 + 96] = lwT[32 * (h_lo + 1):32 * (h_lo + 1) + 32]
    lwPn = np.ascontiguousarray(lwP.astype(bf))
    bias4 = np.zeros((128, 8), np.float32)
    for m in range(2):
        for i, vec in enumerate([Q_bias.reshape(HID), K_bias.reshape(HID),
                                 V_bias.reshape(HID), lin_b.reshape(HID)]):
            bias4[:, 4 * m + i] = vec[128 * m:128 * (m + 1)]
    in_maps = []
    for i in range(NCORES):
        xqT_i = np.ascontiguousarray(
            x_q[i * NQ:(i + 1) * NQ].T.astype(bf))                  # [1024, 512]
        in_maps.append({
            "xkT": x_kT, "xqT": xqT_i, "wq": wq2, "wk": wk2, "wv": wv2,
            "lwP": lwPn, "bias4": bias4,
        })
    return in_maps


def _ensure_ntff_hook():
    """The agent image's antenv lacks axon_hooks; synthesize it from the
    boot script's ctypes NTFF implementation so trace=True works."""
    import types
    try:
        from antenv.axon_hooks import get_axon_ntff_profile_hook  # noqa: F401
        return
    except ImportError:
        pass
    sys.path.insert(0, "/root/.axon_site/trn_agent_boot")
    import trn_boot
    hook = trn_boot._ntff_profile_via_ctypes(
        os.environ.get("PJRT_LIBRARY_PATH", "/opt/axon/libaxon_pjrt.so"))
    mod = types.ModuleType("antenv.axon_hooks")
    mod._hook = hook
    mod.get_axon_ntff_profile_hook = lambda: mod._hook
    mod.set_axon_ntff_profile_hook = lambda h: setattr(mod, "_hook", h)
    sys.modules["antenv.axon_hooks"] = mod


def _run(in_maps, trace=False):
    from concourse.bass_utils import run_bass_kernel_spmd
    if trace:
        _ensure_ntff_hook()
    nc = _get_nc()
    res = run_bass_kernel_spmd(nc, in_maps, core_ids=list(range(NCORES)),
                               trace=trace)
    return res


def kernel(input_x, pe_Q, pe_K, A, WQ, WK, WV, Q_bias, K_bias, V_bias,
           lin_w, lin_b):
    in_maps = _prep_in_maps(
        np.asarray(input_x, np.float32), np.asarray(pe_Q, np.float32),
        np.asarray(pe_K, np.float32), np.asarray(WQ, np.float32),
        np.asarray(WK, np.float32), np.asarray(WV, np.float32),
        np.asarray(Q_bias, np.float32), np.asarray(K_bias, np.float32),
        np.asarray(V_bias, np.float32), np.asarray(lin_w, np.float32),
        np.asarray(lin_b, np.float32))
    res = _run(in_maps)
    out_full = np.empty((N, HID), np.float32)
    for i in range(NCORES):
        out_full[i * NQ:(i + 1) * NQ] = res.results[i]["out"].T
    return out_full


def hw_exec_ns(input_x, pe_Q, pe_K, A, WQ, WK, WV, Q_bias, K_bias, V_bias,
               lin_w, lin_b):
    """Run once with NTFF tracing; returns (exec_time_ns, results)."""
    in_maps = _prep_in_maps(
        np.asarray(input_x, np.float32), np.asarray(pe_Q, np.float32),
        np.asarray(pe_K, np.float32), np.asarray(WQ, np.float32),
        np.asarray(WK, np.float32), np.asarray(WV, np.float32),
        np.asarray(Q_bias, np.float32), np.asarray(K_bias, np.float32),
        np.asarray(V_bias, np.float32), np.asarray(lin_w, np.float32),
        np.asarray(lin_b, np.float32))
    res = _run(in_maps, trace=True)
    return res.exec_time_ns, res


# revision 34
# speedup vs baseline: 1.0114x; 1.0114x over previous
"""Trainium2 Bass kernel for nn_Attention_layer (GNN message passing attention).

Math (see harness reference):
  x_Q = [input_x, pe_Q]  (N, 1024);  x_K = [input_x, pe_K]
  Q = x_Q @ WQ[h] + qb;  K = x_K @ WK[h] + kb;  V = input_x @ WV[h] (+vb=0)
  attn = softmax(Q K^T / 16, axis=k);  out = concat_h(attn @ V) @ lin_w.T + lin_b

Distribution: 8 NeuronCores, query-dim (N) sharded 512 rows/core; K/V work
replicated (no collectives).  Per core, transposed domain (scores^T [k, q]).

Rev B design (head-group-major + PSUM PV accumulation + split exp engines):
  - iteration = (mg, kc): head-group mg in {0,1} (4 heads), k-chunk kc in
    0..31 (128 nodes). All 32 kc of mg=0 first, then mg=1.
  - scores: one 4-bank PSUM tensor st [128, 2048]; 4 matmuls (contraction
    hd=32, row-tiled via tile_position (32j, 0)).
  - exp split across engines by column (rows of a softmax stay on one
    engine so approximation error cancels in Z): ScalarE does true exp on
    cols [0, SPLIT); VectorE does a Schraudolph bit-trick exp on the rest
    (one tensor_scalar affine into int16 bits == bf16 exp, ~1.8% rms,
    zero-mean; confined to 2 of 8 heads -> ~8e-3 output rel err).
  - PV+Z fused: V augmented with a ones column per head (vt [.., 8, 33]);
    one M=33 matmul per head (tile cols 0 / 64) accumulates attn_x^T AND
    the softmax denominator Z directly in a persistent 2-bank PSUM tile
    across all 32 kc (no vector adds, no separate Z matmuls).
  - projections (Q/K/V on PE, bias-add/copy on VectorE; GpSimd cannot
    read PSUM) are interleaved
    just-in-time into the iteration stream, K proj for head-group mg
    scheduled inside mg's own half.
  - epilogue per mg (overlapped with the other half's stream): gather Z
    rows via selector matmul, fast reciprocal, broadcast via outer-product
    matmul, normalize; final linear uses a host-permuted lin_w so the
    PSUM partition layout feeds it directly; lin_w rows are zeroed for
    junk partitions.
"""

import os
import sys
import math
import numpy as np
import ml_dtypes

for _p in ("/opt/trn_rl_repo", "/root/.axon_site/_ro/trn_rl_repo"):
    if os.path.isdir(_p) and _p not in sys.path:
        sys.path.insert(0, _p)

N = 4096
IND = 256          # input_x dim
QKD = 1024         # concat dim for Q/K projections
H = 8              # heads
HD = 32            # head dim
HID = 256          # H * HD
NCORES = 8
NQ = N // NCORES   # 512 query rows per core
SCALE = 1.0 / 16.0  # 1/sqrt(HID)

# exp engine split: ScalarE (true exp) takes score cols [0, SPLIT) = heads
# j0..j2 (tensors stAB+stC); VectorE (Schraudolph) takes head j3 (stD).
SPLIT = 1536
EXP_A = 8.0 / math.log(2.0)        # 128/(16 ln2): bf16-bits slope on raw scores
EXP_B = 16256.0 - 7.4              # 127<<7 minus rms-centering constant

_CACHE = {}


def _build_nc():
    from contextlib import ExitStack
    import concourse.bacc as bacc
    import concourse.tile as tile
    import concourse.mybir as mybir
    from concourse.bass import ds, ts

    f32 = mybir.dt.float32
    bf16 = mybir.dt.bfloat16
    i16 = mybir.dt.int16
    Exp = mybir.ActivationFunctionType.Exp
    mult = mybir.AluOpType.mult
    add = mybir.AluOpType.add

    nc = bacc.Bacc("TRN2", target_bir_lowering=False, debug=False,
                   num_devices=NCORES)

    # Z gather / broadcast selector constants
    selz_np = np.zeros((128, 2), dtype=np.float16)
    selz_np[32, 0] = 1.0
    selz_np[96, 1] = 1.0
    bselm_np = np.zeros((2, 128), dtype=np.float16)
    bselm_np[0, 0:32] = 1.0
    bselm_np[1, 64:96] = 1.0

    # ---- DRAM I/O (per-core shards prepared on host) ----
    xkT = nc.dram_tensor("xkT", [QKD, N], bf16, kind="ExternalInput")   # [x;peK]^T
    xqT = nc.dram_tensor("xqT", [QKD, NQ], bf16, kind="ExternalInput")  # [x;peQ]^T rows blk
    wq = nc.dram_tensor("wq", [QKD, HID], bf16, kind="ExternalInput")   # [d,(h,hd)]
    wk = nc.dram_tensor("wk", [QKD, HID], bf16, kind="ExternalInput")
    wv = nc.dram_tensor("wv", [IND, HID], bf16, kind="ExternalInput")
    lwP = nc.dram_tensor("lwP", [4 * 128, HID], bf16, kind="ExternalInput")  # permuted lin_w.T
    bias4 = nc.dram_tensor("bias4", [128, 8], f32, kind="ExternalInput")  # [p, 4m+i]
    out = nc.dram_tensor("out", [HID, NQ], f32, kind="ExternalOutput")   # out^T


    with tile.TileContext(nc) as tc, ExitStack() as ctx:
        consts = ctx.enter_context(tc.tile_pool(name="consts", bufs=1))
        big = ctx.enter_context(tc.tile_pool(name="big", bufs=1))
        ptp = ctx.enter_context(tc.tile_pool(name="ptp", bufs=6))
        stp = ctx.enter_context(tc.tile_pool(name="stp", bufs=1, space="PSUM"))

        # ---- SBUF tiles ----
        xkt = big.tile([128, 8, N], bf16, tag="xkt")       # x_K^T  (8 c-chunks)
        xqt = big.tile([128, 8, NQ], bf16, tag="xqt")      # x_Q^T block
        wqt = consts.tile([128, 8, HID], bf16, tag="wqt")
        wkt = consts.tile([128, 8, HID], bf16, tag="wkt")
        wvt = consts.tile([128, 2, HID], bf16, tag="wvt")
        lwt = consts.tile([128, 4, HID], bf16, tag="lwt")  # permuted lin_w.T
        bt = consts.tile([128, 8], f32, tag="bt")          # [p, 4m+i]
        selz = consts.tile([128, 2], bf16, tag="selz")
        bselm = consts.tile([2, 128], bf16, tag="bselm")

        kt = big.tile([128, 2, N], bf16, tag="kt")         # K^T rows (h,hd)
        qt = big.tile([128, 2, NQ], bf16, tag="qt")        # Q^T
        vt = big.tile([128, 32, 8, 33], bf16, tag="vt")    # V node-major, +ones col
        pvs = big.tile([128, 2, 2, 512], f32, tag="pvs")   # PV+Z psum copies per mg
        attn2 = big.tile([128, 4, 512], bf16, tag="attn2")  # normalized attn_x^T
        zrm = big.tile([2, 2, 2, NQ], f32, tag="zrm")      # 1/Z [row, mg, b, q]
        zrh = big.tile([2, 2, 2, NQ], bf16, tag="zrh")     # bf16 1/Z for PE
        zsb = big.tile([128, 2, NQ], bf16, tag="zsb")      # bf16 copy of pvz
        outsb = big.tile([128, 2, NQ], f32, tag="outsb")

        # ---- persistent PSUM: separate score tensors per reader engine so
        # WAR chains stay independent (tile dep tracking is per-tensor) ----
        stAB = stp.tile([128, 2, NQ], f32, tag="stAB", name="stAB")  # exp1 (ACT)
        stC = stp.tile([128, NQ], f32, tag="stC", name="stC")        # exp2 (ACT)
        stD = stp.tile([128, NQ], f32, tag="stD", name="stD")        # schr (DVE)
        pvz = stp.tile([128, 2, NQ], f32, tag="pvz", name="pvz")     # 2 banks

        # ---- const / weight DMAs, ordered by first consumer ----
        xkT_r = xkT.rearrange("(c p) (n q) -> n p c q", p=128, q=512)
        xqT_r = xqT.rearrange("(c p) q -> p c q", p=128)
        wq_r = wq.rearrange("(c p) o -> p c o", p=128)
        nc.sync.dma_start(wqt[:, :4], wq_r[:, :4])
        nc.sync.dma_start(xqt[:, :4], xqT_r[:, :4])
        nc.sync.dma_start(wqt[:, 4:], wq_r[:, 4:])
        nc.sync.dma_start(wkt[:], wk.rearrange("(c p) o -> p c o", p=128))
        nc.sync.dma_start(xkt[:, :, ds(0, 128)], xkT_r[0][:, :, ds(0, 128)])
        nc.sync.dma_start(xqt[:, 4:], xqT_r[:, 4:])
        nc.sync.dma_start(bt[:], bias4[:])
        nc.sync.dma_start(wvt[:], wv.rearrange("(c p) o -> p c o", p=128))
        nc.sync.dma_start(xkt[:, :, ds(128, 384)], xkT_r[0][:, :, ds(128, 384)])
        for n in range(1, 8):
            nc.sync.dma_start(xkt[:, :, ts(n, 512)], xkT_r[n])
        nc.sync.dma_start(lwt[:], lwP.rearrange("(c p) o -> p c o", p=128))
        nc.sync.dma_start(selz[:], nc.inline_tensor(
            selz_np.astype(ml_dtypes.bfloat16), name="selz_c")[:])
        nc.sync.dma_start(bselm[:], nc.inline_tensor(
            bselm_np.astype(ml_dtypes.bfloat16), name="bselm_c")[:])

        # PE clock warmup: dependency-free dummy matmuls on a locally
        # memset tile start right after the preamble and ramp the PE p-state
        # while the input DMAs land.
        warm = big.tile([128, 512], bf16, tag="warm")
        nc.vector.memset(warm[:], 1.0)
        for w in range(8):
            nc.tensor.matmul(stAB[:, 0, :], warm[:, ds(0, 128)], warm[:],
                             start=True, stop=True)
        # preload the ACT exp table set while DMAs land
        actwarm = consts.tile([8, 16], f32, tag="actwarm")
        nc.vector.memset(actwarm[:], 0.0)
        nc.scalar.activation(actwarm[:], actwarm[:], Exp)
        # ones-column of augmented V (never overwritten: V copies skip col 32)
        for h in range(H):
            nc.vector.memset(vt[:, :, h, ds(32, 1)], 1.0)

        # PV+Z accumulates with start=False onto explicitly zeroed banks
        # (two start=True matmuls sharing a bank would re-zero each other)
        nc.vector.memset(pvz[:], 0.0)

        # ---- projection helpers (PE matmuls + GpSimd bias-add/copy) ----
        def q_proj_unit(m, lo, nmm):
            ps = stp.tile([128, NQ], f32, tag="pzv", bufs=1, name=f"qp{m}_{lo}")
            for c in range(lo, lo + nmm):
                nc.tensor.matmul(ps[:, :NQ], wqt[:, c, ts(m, 128)], xqt[:, c, :],
                                 start=(c == 0), stop=(c == 7))
            if lo + nmm == 8:
                nc.vector.tensor_scalar_add(qt[:, m, :], ps[:, :NQ],
                                            bt[:, 4 * m + 0:4 * m + 1])
            return ps

        qproj_open = {}

        def q_proj(m, half):
            if half == 0:
                qproj_open[m] = q_proj_unit(m, 0, 4)
            else:
                ps = qproj_open.pop(m)
                for c in range(4, 8):
                    nc.tensor.matmul(ps[:, :NQ], wqt[:, c, ts(m, 128)],
                                     xqt[:, c, :], start=False, stop=(c == 7))
                nc.vector.tensor_scalar_add(qt[:, m, :], ps[:, :NQ],
                                            bt[:, 4 * m + 0:4 * m + 1])

        def k_proj_narrow(n, m, lo, w):
            ps = stp.tile([128, NQ], f32, tag="pzk", bufs=1, name=f"kn{n}_{m}_{lo}")
            for c in range(8):
                nc.tensor.matmul(ps[:, :w], wkt[:, c, ts(m, 128)],
                                 xkt[:, c, ds(512 * n + lo, w)],
                                 start=(c == 0), stop=(c == 7))
            nc.vector.tensor_scalar_add(kt[:, m, ds(512 * n + lo, w)], ps[:, :w],
                                        bt[:, 4 * m + 1:4 * m + 2])

        kproj_open = {}

        def k_proj_quarter(n, m, qtr):
            if qtr == 0:
                ps = stp.tile([128, NQ], f32, tag="pzk", bufs=1, name=f"kp{n}_{m}")
                kproj_open[(n, m)] = ps
            else:
                ps = kproj_open[(n, m)]
            for c in range(2 * qtr, 2 * qtr + 2):
                nc.tensor.matmul(ps[:, :512], wkt[:, c, ts(m, 128)],
                                 xkt[:, c, ts(n, 512)],
                                 start=(c == 0), stop=(c == 7))
            if qtr == 3:
                del kproj_open[(n, m)]
                nc.vector.tensor_scalar_add(kt[:, m, ts(n, 512)], ps[:, :512],
                                            bt[:, 4 * m + 1:4 * m + 2])

        def v_proj_unit(kc):
            ps = stp.tile([128, NQ], f32, tag="pzv", bufs=1, name=f"vp{kc}")
            for c in range(2):
                nc.tensor.matmul(ps[:, :HID], xkt[:, c, ds(128 * kc, 128)],
                                 wvt[:, c, :], start=(c == 0), stop=(c == 1))
            # strided copy into augmented V layout (skips the ones column)
            nc.vector.tensor_copy(out=vt[:, kc, :, ds(0, 32)], in_=ps[:, :HID])

        # ---- epilogue per head-group ----
        def epilogue_zsb(b):
            nc.vector.tensor_copy(out=zsb[:, b, :], in_=pvz[:, b])

        def epilogue_copy(mg, b):
            nc.vector.tensor_copy(out=pvs[:, mg, b], in_=pvz[:, b])

        def pvz_clear():
            nc.vector.memset(pvz[:], 0.0)

        def epilogue_zq(mg, b):
            zq = stp.tile([128, NQ], f32, tag="pzv", bufs=1, name=f"zq{mg}_{b}")
            nc.tensor.matmul(zq[ds(0, 2), :NQ], selz[:], zsb[:, b, :],
                             start=True, stop=True)
            nc.vector.reciprocal_approx_fast(zrm[ds(0, 2), mg, b, :],
                                             zq[ds(0, 2), :NQ])
            nc.vector.tensor_copy(out=zrh[ds(0, 2), mg, b, :],
                                  in_=zrm[ds(0, 2), mg, b, :])

        def epilogue_norm(mg, b):
            psb = stp.tile([128, NQ], f32, tag="pzv", bufs=1, name=f"psb{mg}_{b}")
            nc.tensor.matmul(psb[:, :NQ], bselm[:], zrh[:, mg, b, :],
                             start=True, stop=True)
            nc.vector.tensor_tensor(attn2[:, 2 * mg + b, :], pvs[:, mg, b, :],
                                    psb[:, :NQ], mult)

        # ---- minimal prologue: what iteration 0 needs ----
        q_proj(0, 0)
        q_proj(0, 1)
        k_proj_narrow(0, 0, 0, 128)
        v_proj_unit(0)
        k_proj_narrow(0, 0, 128, 384)

        # ---- interleaved work schedule: slot i = iteration i ----
        pre_work = {}
        post_work = {}

        def at(i, fn):
            pre_work.setdefault(i, []).append(fn)

        def at_post(i, fn):
            post_work.setdefault(i, []).append(fn)

        # K proj m=0 tiles 1..7 during the preceding 4 iterations
        for n in range(1, 8):
            for qtr in range(4):
                at(max(1, 4 * (n - 1) + qtr),
                   lambda n=n, qtr=qtr: k_proj_quarter(n, 0, qtr))
        # K proj m=1 tile 0 late in the first half; tiles 1..7 in second half
        at(28, lambda: k_proj_narrow(0, 1, 0, 256))
        at(30, lambda: k_proj_narrow(0, 1, 256, 256))
        for n in range(1, 8):
            for qtr in range(4):
                at(32 + 4 * (n - 1) + qtr,
                   lambda n=n, qtr=qtr: k_proj_quarter(n, 1, qtr))
        # V chunks a couple of iterations ahead of their PV use
        for kc in range(1, 32):
            at(max(0, kc - 2), lambda kc=kc: v_proj_unit(kc))
        # Q proj m=1 before the second half
        at(22, lambda: q_proj(1, 0))
        at(26, lambda: q_proj(1, 1))
        # head-group 0 epilogue, overlapped with the mg=1 stream
        at_post(33, lambda: epilogue_zsb(0))
        at_post(33, lambda: epilogue_zsb(1))
        at_post(34, lambda: epilogue_copy(0, 0))
        at_post(34, lambda: epilogue_zq(0, 0))
        at_post(35, lambda: epilogue_copy(0, 1))
        at_post(35, lambda: epilogue_zq(0, 1))
        at_post(35, lambda: pvz_clear())
        at_post(36, lambda: epilogue_norm(0, 0))
        at_post(37, lambda: epilogue_norm(0, 1))

        # ---- PV+Z unit: matmuls accumulating in persistent PSUM ----
        def pvz_unit(ptA, ptB, mg, kc):
            for b in range(2):
                for half in range(2):
                    j = 2 * b + half
                    h = 4 * mg + j
                    rhs = (ptA[:, j, :] if j < 3 else ptB[:].bitcast(bf16))
                    nc.tensor.matmul(
                        pvz[ds(64 * half, 33), b, :],
                        vt[:, kc, h, :], rhs,
                        start=False, stop=(kc == 31),
                        tile_position=(0, 64 * half),
                        skip_group_check=True)

        # ---- main loop: 64 iterations of (mg, kc) ----
        pending = []
        for i in range(64):
            mg, kc = divmod(i, 32)[0], i % 32
            for fn in pre_work.get(i, []):
                fn()
            ptA = ptp.tile([128, 3, NQ], bf16, tag="ptA", name="ptA")
            ptB = ptp.tile([128, NQ], i16, tag="ptB", name="ptB")
            sdst = [stAB[:, 0, :], stAB[:, 1, :], stC[:, :], stD[:, :]]
            for j in range(4):
                nc.tensor.matmul(
                    sdst[j],
                    kt[ds(32 * j, 32), mg, ds(128 * kc, 128)],
                    qt[ds(32 * j, 32), mg, :],
                    start=True, stop=True,
                    tile_position=(32 * j, 0))
            nc.scalar.activation(ptA[:, 0:2, :], stAB[:], Exp, scale=SCALE)
            nc.scalar.activation(ptA[:, 2, :], stC[:], Exp, scale=SCALE)
            nc.vector.tensor_scalar(ptB[:], stD[:], EXP_A, EXP_B, mult, add)
            # hold mg=1's first PV units two extra slots so the mg=0
            # epilogue (which must finish reading/clearing pvz first) can
            # spread out; catch back up with double flushes.
            n_flush = {34: 0, 35: 0, 36: 2, 37: 2}.get(i, 1)
            for _ in range(n_flush):
                if len(pending) >= 2:
                    pvz_unit(*pending.pop(0))
            pending.append((ptA, ptB, mg, kc))
            for fn in post_work.get(i, []):
                fn()
        for args in pending:
            pvz_unit(*args)

        # ---- tail: head-group 1 epilogue + final linear (lin reuses the
        # dead score PSUM banks; per-bank chains interleave) ----
        def tdummy(n=1):
            for _ in range(n):
                nc.tensor.matmul(stAB[:, 0, :], warm[:, ds(0, 128)], warm[:],
                                 start=True, stop=True)

        epilogue_zsb(0)
        epilogue_zsb(1)
        # final linear: mg=0 chunks accumulate as soon as the score banks die
        for mo, ps in ((0, stC), (1, stD)):
            for c in range(2):
                nc.tensor.matmul(ps[:, :NQ], lwt[:, c, ts(mo, 128)],
                                 attn2[:, c, :], start=(c == 0), stop=False)
        epilogue_zq(1, 0)
        epilogue_copy(1, 0)
        epilogue_zq(1, 1)
        tdummy(2)
        epilogue_copy(1, 1)
        epilogue_norm(1, 0)
        for mo, ps in ((0, stC), (1, stD)):
            nc.tensor.matmul(ps[:, :NQ], lwt[:, 2, ts(mo, 128)],
                             attn2[:, 2, :], start=False, stop=False)
        epilogue_norm(1, 1)
        out_r = out.rearrange("(m p) q -> p m q", p=128)
        for mo, ps in ((0, stC), (1, stD)):
            nc.tensor.matmul(ps[:, :NQ], lwt[:, 3, ts(mo, 128)],
                             attn2[:, 3, :], start=False, stop=True)
            nc.vector.tensor_scalar_add(outsb[:, mo, :], ps[:, :NQ],
                                        bt[:, 4 * mo + 3:4 * mo + 4])
            nc.sync.dma_start(out_r[:, mo], outsb[:, mo, :])

    nc.compile()
    return nc


def _get_nc():
    if "nc" not in _CACHE:
        _CACHE["nc"] = _build_nc()
    return _CACHE["nc"]


def _prep_in_maps(input_x, pe_Q, pe_K, WQ, WK, WV, Q_bias, K_bias, V_bias,
                  lin_w, lin_b):
    bf = ml_dtypes.bfloat16
    x_kT = np.ascontiguousarray(
        np.concatenate([input_x, pe_K], axis=1).T.astype(bf))       # [1024, 4096]
    x_q = np.concatenate([input_x, pe_Q], axis=1)                   # [4096, 1024]
    wq2 = np.ascontiguousarray(
        WQ.transpose(1, 0, 2).reshape(QKD, HID).astype(bf))         # [d,(h,hd)]
    wk2 = np.ascontiguousarray(WK.transpose(1, 0, 2).reshape(QKD, HID).astype(bf))
    wv2 = np.ascontiguousarray(WV.transpose(1, 0, 2).reshape(IND, HID).astype(bf))
    # permuted lin_w.T for the PSUM partition layout: chunk c = 2*mg + b,
    # partition p<32 -> head (4mg+2b) row p; 64<=p<96 -> head (4mg+2b+1)
    # row p-64; other partitions (Z rows + junk) get zero weights.
    lwT = lin_w.T  # [HID_in (h, hd), HID_out]
    lwP = np.zeros((4 * 128, HID), np.float32)
    for c in range(4):
        mg, b = divmod(c, 2)
        h_lo = 4 * mg + 2 * b
        lwP[c * 128 + 0:c * 128 + 32] = lwT[32 * h_lo:32 * h_lo + 32]
        lwP[c * 128 + 64:c * 128 + 96] = lwT[32 * (h_lo + 1):32 * (h_lo + 1) + 32]
    lwPn = np.ascontiguousarray(lwP.astype(bf))
    bias4 = np.zeros((128, 8), np.float32)
    for m in range(2):
        for i, vec in enumerate([Q_bias.reshape(HID), K_bias.reshape(HID),
                                 V_bias.reshape(HID), lin_b.reshape(HID)]):
            bias4[:, 4 * m + i] = vec[128 * m:128 * (m + 1)]
    in_maps = []
    for i in range(NCORES):
        xqT_i = np.ascontiguousarray(
            x_q[i * NQ:(i + 1) * NQ].T.astype(bf))                  # [1024, 512]
        in_maps.append({
            "xkT": x_kT, "xqT": xqT_i, "wq": wq2, "wk": wk2, "wv": wv2,
            "lwP": lwPn, "bias4": bias4,
        })
    return in_maps


def _ensure_ntff_hook():
    """The agent image's antenv lacks axon_hooks; synthesize it from the
    boot script's ctypes NTFF implementation so trace=True works."""
    import types
    try:
        from antenv.axon_hooks import get_axon_ntff_profile_hook  # noqa: F401
        return
    except ImportError:
        pass
    sys.path.insert(0, "/root/.axon_site/trn_agent_boot")
    import trn_boot
    hook = trn_boot._ntff_profile_via_ctypes(
        os.environ.get("PJRT_LIBRARY_PATH", "/opt/axon/libaxon_pjrt.so"))
    mod = types.ModuleType("antenv.axon_hooks")
    mod._hook = hook
    mod.get_axon_ntff_profile_hook = lambda: mod._hook
    mod.set_axon_ntff_profile_hook = lambda h: setattr(mod, "_hook", h)
    sys.modules["antenv.axon_hooks"] = mod


def _run(in_maps, trace=False):
    from concourse.bass_utils import run_bass_kernel_spmd
    if trace:
        _ensure_ntff_hook()
    nc = _get_nc()
    res = run_bass_kernel_spmd(nc, in_maps, core_ids=list(range(NCORES)),
                               trace=trace)
    return res


def kernel(input_x, pe_Q, pe_K, A, WQ, WK, WV, Q_bias, K_bias, V_bias,
           lin_w, lin_b):
    in_maps = _prep_in_maps(
        np.asarray(input_x, np.float32), np.asarray(pe_Q, np.float32),
        np.asarray(pe_K, np.float32), np.asarray(WQ, np.float32),
        np.asarray(WK, np.float32), np.asarray(WV, np.float32),
        np.asarray(Q_bias, np.float32), np.asarray(K_bias, np.float32),
        np.asarray(V_bias, np.float32), np.asarray(lin_w, np.float32),
        np.asarray(lin_b, np.float32))
    res = _run(in_maps)
    out_full = np.empty((N, HID), np.float32)
    for i in range(NCORES):
        out_full[i * NQ:(i + 1) * NQ] = res.results[i]["out"].T
    return out_full


def hw_exec_ns(input_x, pe_Q, pe_K, A, WQ, WK, WV, Q_bias, K_bias, V_bias,
               lin_w, lin_b):
    """Run once with NTFF tracing; returns (exec_time_ns, results)."""
    in_maps = _prep_in_maps(
        np.asarray(input_x, np.float32), np.asarray(pe_Q, np.float32),
        np.asarray(pe_K, np.float32), np.asarray(WQ, np.float32),
        np.asarray(WK, np.float32), np.asarray(WV, np.float32),
        np.asarray(Q_bias, np.float32), np.asarray(K_bias, np.float32),
        np.asarray(V_bias, np.float32), np.asarray(lin_w, np.float32),
        np.asarray(lin_b, np.float32))
    res = _run(in_maps, trace=True)
    return res.exec_time_ns, res


# revision 35
# speedup vs baseline: 1.0247x; 1.0131x over previous
"""Trainium2 Bass kernel for nn_Attention_layer (GNN message passing attention).

Math (see harness reference):
  x_Q = [input_x, pe_Q]  (N, 1024);  x_K = [input_x, pe_K]
  Q = x_Q @ WQ[h] + qb;  K = x_K @ WK[h] + kb;  V = input_x @ WV[h] (+vb=0)
  attn = softmax(Q K^T / 16, axis=k);  out = concat_h(attn @ V) @ lin_w.T + lin_b

Distribution: 8 NeuronCores, query-dim (N) sharded 512 rows/core; K/V work
replicated (no collectives).  Per core, transposed domain (scores^T [k, q]).

Rev B design (head-group-major + PSUM PV accumulation + split exp engines):
  - iteration = (mg, kc): head-group mg in {0,1} (4 heads), k-chunk kc in
    0..31 (128 nodes). All 32 kc of mg=0 first, then mg=1.
  - scores: one 4-bank PSUM tensor st [128, 2048]; 4 matmuls (contraction
    hd=32, row-tiled via tile_position (32j, 0)).
  - exp split across engines by column (rows of a softmax stay on one
    engine so approximation error cancels in Z): ScalarE does true exp on
    cols [0, SPLIT); VectorE does a Schraudolph bit-trick exp on the rest
    (one tensor_scalar affine into int16 bits == bf16 exp, ~1.8% rms,
    zero-mean; confined to 2 of 8 heads -> ~8e-3 output rel err).
  - PV+Z fused: V augmented with a ones column per head (vt [.., 8, 33]);
    one M=33 matmul per head (tile cols 0 / 64) accumulates attn_x^T AND
    the softmax denominator Z directly in a persistent 2-bank PSUM tile
    across all 32 kc (no vector adds, no separate Z matmuls).
  - projections (Q/K/V on PE, bias-add/copy on VectorE; GpSimd cannot
    read PSUM) are interleaved
    just-in-time into the iteration stream, K proj for head-group mg
    scheduled inside mg's own half.
  - epilogue per mg (overlapped with the other half's stream): gather Z
    rows via selector matmul, fast reciprocal, broadcast via outer-product
    matmul, normalize; final linear uses a host-permuted lin_w so the
    PSUM partition layout feeds it directly; lin_w rows are zeroed for
    junk partitions.
"""

import os
import sys
import math
import numpy as np
import ml_dtypes

for _p in ("/opt/trn_rl_repo", "/root/.axon_site/_ro/trn_rl_repo"):
    if os.path.isdir(_p) and _p not in sys.path:
        sys.path.insert(0, _p)

N = 4096
IND = 256          # input_x dim
QKD = 1024         # concat dim for Q/K projections
H = 8              # heads
HD = 32            # head dim
HID = 256          # H * HD
NCORES = 8
NQ = N // NCORES   # 512 query rows per core
SCALE = 1.0 / 16.0  # 1/sqrt(HID)

# exp engine split: ScalarE (true exp) takes score cols [0, SPLIT) = heads
# j0..j2 (tensors stAB+stC); VectorE (Schraudolph) takes head j3 (stD).
SPLIT = 1536
EXP_A = 8.0 / math.log(2.0)        # 128/(16 ln2): bf16-bits slope on raw scores
EXP_B = 16256.0 - 7.4              # 127<<7 minus rms-centering constant

_CACHE = {}


def _build_nc():
    from contextlib import ExitStack
    import concourse.bacc as bacc
    import concourse.tile as tile
    import concourse.mybir as mybir
    from concourse.bass import ds, ts

    f32 = mybir.dt.float32
    bf16 = mybir.dt.bfloat16
    i16 = mybir.dt.int16
    Exp = mybir.ActivationFunctionType.Exp
    mult = mybir.AluOpType.mult
    add = mybir.AluOpType.add

    nc = bacc.Bacc("TRN2", target_bir_lowering=False, debug=False,
                   num_devices=NCORES)

    # Z gather / broadcast selector constants
    selz_np = np.zeros((128, 2), dtype=np.float16)
    selz_np[32, 0] = 1.0
    selz_np[96, 1] = 1.0
    bselm_np = np.zeros((2, 128), dtype=np.float16)
    bselm_np[0, 0:32] = 1.0
    bselm_np[1, 64:96] = 1.0

    # ---- DRAM I/O (per-core shards prepared on host) ----
    xkT = nc.dram_tensor("xkT", [QKD, N], bf16, kind="ExternalInput")   # [x;peK]^T
    xqT = nc.dram_tensor("xqT", [QKD, NQ], bf16, kind="ExternalInput")  # [x;peQ]^T rows blk
    wq = nc.dram_tensor("wq", [QKD, HID], bf16, kind="ExternalInput")   # [d,(h,hd)]
    wk = nc.dram_tensor("wk", [QKD, HID], bf16, kind="ExternalInput")
    wv = nc.dram_tensor("wv", [IND, HID], bf16, kind="ExternalInput")
    lwP = nc.dram_tensor("lwP", [4 * 128, HID], bf16, kind="ExternalInput")  # permuted lin_w.T
    bias4 = nc.dram_tensor("bias4", [128, 8], f32, kind="ExternalInput")  # [p, 4m+i]
    out = nc.dram_tensor("out", [HID, NQ], f32, kind="ExternalOutput")   # out^T


    with tile.TileContext(nc) as tc, ExitStack() as ctx:
        consts = ctx.enter_context(tc.tile_pool(name="consts", bufs=1))
        big = ctx.enter_context(tc.tile_pool(name="big", bufs=1))
        ptp = ctx.enter_context(tc.tile_pool(name="ptp", bufs=6))
        stp = ctx.enter_context(tc.tile_pool(name="stp", bufs=1, space="PSUM"))

        # ---- SBUF tiles ----
        xkt = big.tile([128, 8, N], bf16, tag="xkt")       # x_K^T  (8 c-chunks)
        xqt = big.tile([128, 8, NQ], bf16, tag="xqt")      # x_Q^T block
        wqt = consts.tile([128, 8, HID], bf16, tag="wqt")
        wkt = consts.tile([128, 8, HID], bf16, tag="wkt")
        wvt = consts.tile([128, 2, HID], bf16, tag="wvt")
        lwt = consts.tile([128, 4, HID], bf16, tag="lwt")  # permuted lin_w.T
        bt = consts.tile([128, 8], f32, tag="bt")          # [p, 4m+i]
        selz = consts.tile([128, 2], bf16, tag="selz")
        bselm = consts.tile([2, 128], bf16, tag="bselm")

        kt = big.tile([128, 2, N], bf16, tag="kt")         # K^T rows (h,hd)
        qt = big.tile([128, 2, NQ], bf16, tag="qt")        # Q^T
        vt = big.tile([128, 32, 8, 33], bf16, tag="vt")    # V node-major, +ones col
        pvs = big.tile([128, 2, 2, 512], f32, tag="pvs")   # PV+Z psum copies per mg
        attn2 = big.tile([128, 4, 512], bf16, tag="attn2")  # normalized attn_x^T
        zrm = big.tile([2, 2, 2, NQ], f32, tag="zrm")      # 1/Z [row, mg, b, q]
        zrh = big.tile([2, 2, 2, NQ], bf16, tag="zrh")     # bf16 1/Z for PE
        zsb = big.tile([128, 2, NQ], bf16, tag="zsb")      # bf16 copy of pvz
        outsb = big.tile([128, 2, NQ], f32, tag="outsb")

        # ---- persistent PSUM: separate score tensors per reader engine so
        # WAR chains stay independent (tile dep tracking is per-tensor) ----
        stAB = stp.tile([128, 2, NQ], f32, tag="stAB", name="stAB")  # exp1 (ACT)
        stC = stp.tile([128, NQ], f32, tag="stC", name="stC")        # exp2 (ACT)
        stD = stp.tile([128, NQ], f32, tag="stD", name="stD")        # schr (DVE)
        pvz = stp.tile([128, 2, NQ], f32, tag="pvz", name="pvz")     # 2 banks

        # ---- const / weight DMAs, ordered by first consumer ----
        xkT_r = xkT.rearrange("(c p) (n q) -> n p c q", p=128, q=512)
        xqT_r = xqT.rearrange("(c p) q -> p c q", p=128)
        nc.sync.dma_start(wqt[:], wq.rearrange("(c p) o -> p c o", p=128))
        nc.sync.dma_start(xqt[:, :4], xqT_r[:, :4])
        nc.sync.dma_start(wkt[:], wk.rearrange("(c p) o -> p c o", p=128))
        nc.sync.dma_start(xkt[:, :, ds(0, 128)], xkT_r[0][:, :, ds(0, 128)])
        nc.sync.dma_start(xqt[:, 4:], xqT_r[:, 4:])
        nc.sync.dma_start(bt[:], bias4[:])
        nc.sync.dma_start(wvt[:], wv.rearrange("(c p) o -> p c o", p=128))
        nc.sync.dma_start(xkt[:, :, ds(128, 384)], xkT_r[0][:, :, ds(128, 384)])
        for n in range(1, 8):
            nc.sync.dma_start(xkt[:, :, ts(n, 512)], xkT_r[n])
        nc.sync.dma_start(lwt[:], lwP.rearrange("(c p) o -> p c o", p=128))
        nc.sync.dma_start(selz[:], nc.inline_tensor(
            selz_np.astype(ml_dtypes.bfloat16), name="selz_c")[:])
        nc.sync.dma_start(bselm[:], nc.inline_tensor(
            bselm_np.astype(ml_dtypes.bfloat16), name="bselm_c")[:])

        # PE clock warmup: dependency-free dummy matmuls on a locally
        # memset tile start right after the preamble and ramp the PE p-state
        # while the input DMAs land.
        warm = big.tile([128, 512], bf16, tag="warm")
        nc.vector.memset(warm[:], 1.0)
        for w in range(8):
            nc.tensor.matmul(stAB[:, 0, :], warm[:, ds(0, 128)], warm[:],
                             start=True, stop=True)
        # preload the ACT exp table set while DMAs land
        actwarm = consts.tile([8, 16], f32, tag="actwarm")
        nc.vector.memset(actwarm[:], 0.0)
        nc.scalar.activation(actwarm[:], actwarm[:], Exp)
        # ones-column of augmented V (never overwritten: V copies skip col 32)
        for h in range(H):
            nc.vector.memset(vt[:, :, h, ds(32, 1)], 1.0)

        # PV+Z accumulates with start=False onto explicitly zeroed banks
        # (two start=True matmuls sharing a bank would re-zero each other)
        nc.vector.memset(pvz[:], 0.0)

        # ---- projection helpers (PE matmuls + GpSimd bias-add/copy) ----
        def q_proj_unit(m, lo, nmm):
            ps = stp.tile([128, NQ], f32, tag="pzv", bufs=1, name=f"qp{m}_{lo}")
            for c in range(lo, lo + nmm):
                nc.tensor.matmul(ps[:, :NQ], wqt[:, c, ts(m, 128)], xqt[:, c, :],
                                 start=(c == 0), stop=(c == 7))
            if lo + nmm == 8:
                nc.vector.tensor_scalar_add(qt[:, m, :], ps[:, :NQ],
                                            bt[:, 4 * m + 0:4 * m + 1])
            return ps

        qproj_open = {}

        def q_proj(m, half):
            if half == 0:
                qproj_open[m] = q_proj_unit(m, 0, 4)
            else:
                ps = qproj_open.pop(m)
                for c in range(4, 8):
                    nc.tensor.matmul(ps[:, :NQ], wqt[:, c, ts(m, 128)],
                                     xqt[:, c, :], start=False, stop=(c == 7))
                nc.vector.tensor_scalar_add(qt[:, m, :], ps[:, :NQ],
                                            bt[:, 4 * m + 0:4 * m + 1])

        def k_proj_narrow(n, m, lo, w):
            ps = stp.tile([128, NQ], f32, tag="pzk", bufs=1, name=f"kn{n}_{m}_{lo}")
            for c in range(8):
                nc.tensor.matmul(ps[:, :w], wkt[:, c, ts(m, 128)],
                                 xkt[:, c, ds(512 * n + lo, w)],
                                 start=(c == 0), stop=(c == 7))
            nc.vector.tensor_scalar_add(kt[:, m, ds(512 * n + lo, w)], ps[:, :w],
                                        bt[:, 4 * m + 1:4 * m + 2])

        kproj_open = {}

        def k_proj_quarter(n, m, qtr):
            if qtr == 0:
                ps = stp.tile([128, NQ], f32, tag="pzk", bufs=1, name=f"kp{n}_{m}")
                kproj_open[(n, m)] = ps
            else:
                ps = kproj_open[(n, m)]
            for c in range(2 * qtr, 2 * qtr + 2):
                nc.tensor.matmul(ps[:, :512], wkt[:, c, ts(m, 128)],
                                 xkt[:, c, ts(n, 512)],
                                 start=(c == 0), stop=(c == 7))
            if qtr == 3:
                del kproj_open[(n, m)]
                nc.vector.tensor_scalar_add(kt[:, m, ts(n, 512)], ps[:, :512],
                                            bt[:, 4 * m + 1:4 * m + 2])

        def v_proj_unit(kc):
            ps = stp.tile([128, NQ], f32, tag="pzv", bufs=1, name=f"vp{kc}")
            for c in range(2):
                nc.tensor.matmul(ps[:, :HID], xkt[:, c, ds(128 * kc, 128)],
                                 wvt[:, c, :], start=(c == 0), stop=(c == 1))
            # strided copy into augmented V layout (skips the ones column)
            nc.vector.tensor_copy(out=vt[:, kc, :, ds(0, 32)], in_=ps[:, :HID])

        # ---- epilogue per head-group ----
        def epilogue_zsb(b):
            nc.vector.tensor_copy(out=zsb[:, b, :], in_=pvz[:, b])

        def epilogue_copy(mg, b):
            nc.vector.tensor_copy(out=pvs[:, mg, b], in_=pvz[:, b])

        def pvz_clear():
            nc.vector.memset(pvz[:], 0.0)

        def epilogue_zq(mg, b):
            zq = stp.tile([128, NQ], f32, tag="pzv", bufs=1, name=f"zq{mg}_{b}")
            nc.tensor.matmul(zq[ds(0, 2), :NQ], selz[:], zsb[:, b, :],
                             start=True, stop=True)
            nc.vector.reciprocal_approx_fast(zrm[ds(0, 2), mg, b, :],
                                             zq[ds(0, 2), :NQ])
            nc.vector.tensor_copy(out=zrh[ds(0, 2), mg, b, :],
                                  in_=zrm[ds(0, 2), mg, b, :])

        def epilogue_norm(mg, b):
            psb = stp.tile([128, NQ], f32, tag="pzv", bufs=1, name=f"psb{mg}_{b}")
            nc.tensor.matmul(psb[:, :NQ], bselm[:], zrh[:, mg, b, :],
                             start=True, stop=True)
            nc.vector.tensor_tensor(attn2[:, 2 * mg + b, :], pvs[:, mg, b, :],
                                    psb[:, :NQ], mult)

        # ---- minimal prologue: what iteration 0 needs ----
        q_proj(0, 0)
        q_proj(0, 1)
        k_proj_narrow(0, 0, 0, 128)
        v_proj_unit(0)
        k_proj_narrow(0, 0, 128, 384)

        # ---- interleaved work schedule: slot i = iteration i ----
        pre_work = {}
        post_work = {}

        def at(i, fn):
            pre_work.setdefault(i, []).append(fn)

        def at_post(i, fn):
            post_work.setdefault(i, []).append(fn)

        # K proj m=0 tiles 1..7 during the preceding 4 iterations
        for n in range(1, 8):
            for qtr in range(4):
                at(max(1, 4 * (n - 1) + qtr),
                   lambda n=n, qtr=qtr: k_proj_quarter(n, 0, qtr))
        # K proj m=1 tile 0 late in the first half; tiles 1..7 in second half
        at(28, lambda: k_proj_narrow(0, 1, 0, 256))
        at(30, lambda: k_proj_narrow(0, 1, 256, 256))
        for n in range(1, 8):
            for qtr in range(4):
                at(32 + 4 * (n - 1) + qtr,
                   lambda n=n, qtr=qtr: k_proj_quarter(n, 1, qtr))
        # V chunks a couple of iterations ahead of their PV use
        for kc in range(1, 32):
            at(max(0, kc - 2), lambda kc=kc: v_proj_unit(kc))
        # Q proj m=1 before the second half
        at(22, lambda: q_proj(1, 0))
        at(26, lambda: q_proj(1, 1))
        # head-group 0 epilogue, overlapped with the mg=1 stream
        at_post(33, lambda: epilogue_zsb(0))
        at_post(33, lambda: epilogue_zsb(1))
        at_post(34, lambda: epilogue_copy(0, 0))
        at_post(34, lambda: epilogue_zq(0, 0))
        at_post(35, lambda: epilogue_copy(0, 1))
        at_post(35, lambda: epilogue_zq(0, 1))
        at_post(35, lambda: pvz_clear())
        at_post(36, lambda: epilogue_norm(0, 0))
        at_post(37, lambda: epilogue_norm(0, 1))

        # ---- PV+Z unit: matmuls accumulating in persistent PSUM ----
        def pvz_unit(ptA, ptB, mg, kc):
            for b in range(2):
                for half in range(2):
                    j = 2 * b + half
                    h = 4 * mg + j
                    rhs = (ptA[:, j, :] if j < 3 else ptB[:].bitcast(bf16))
                    nc.tensor.matmul(
                        pvz[ds(64 * half, 33), b, :],
                        vt[:, kc, h, :], rhs,
                        start=False, stop=(kc == 31),
                        tile_position=(0, 64 * half),
                        skip_group_check=True)

        # ---- main loop: 64 iterations of (mg, kc) ----
        pending = []
        for i in range(64):
            mg, kc = divmod(i, 32)[0], i % 32
            for fn in pre_work.get(i, []):
                fn()
            ptA = ptp.tile([128, 3, NQ], bf16, tag="ptA", name="ptA")
            ptB = ptp.tile([128, NQ], i16, tag="ptB", name="ptB")
            sdst = [stAB[:, 0, :], stAB[:, 1, :], stC[:, :], stD[:, :]]
            for j in range(4):
                nc.tensor.matmul(
                    sdst[j],
                    kt[ds(32 * j, 32), mg, ds(128 * kc, 128)],
                    qt[ds(32 * j, 32), mg, :],
                    start=True, stop=True,
                    tile_position=(32 * j, 0))
            nc.scalar.activation(ptA[:, 0:2, :], stAB[:], Exp, scale=SCALE)
            nc.scalar.activation(ptA[:, 2, :], stC[:], Exp, scale=SCALE)
            nc.vector.tensor_scalar(ptB[:], stD[:], EXP_A, EXP_B, mult, add)
            # hold mg=1's first PV units two extra slots so the mg=0
            # epilogue (which must finish reading/clearing pvz first) can
            # spread out; catch back up with double flushes.
            n_flush = {34: 0, 35: 0, 36: 2, 37: 2}.get(i, 1)
            for _ in range(n_flush):
                if len(pending) >= 2:
                    pvz_unit(*pending.pop(0))
            pending.append((ptA, ptB, mg, kc))
            for fn in post_work.get(i, []):
                fn()
        for args in pending:
            pvz_unit(*args)

        # ---- tail: head-group 1 epilogue + final linear (lin reuses the
        # dead score PSUM banks; per-bank chains interleave) ----
        def tdummy(n=1):
            for _ in range(n):
                nc.tensor.matmul(stAB[:, 0, :], warm[:, ds(0, 128)], warm[:],
                                 start=True, stop=True)

        epilogue_zsb(0)
        epilogue_zsb(1)
        epilogue_zq(1, 0)
        epilogue_copy(1, 0)
        epilogue_zq(1, 1)
        tdummy(2)
        epilogue_copy(1, 1)
        epilogue_norm(1, 0)
        tdummy(2)
        epilogue_norm(1, 1)
        tdummy(2)
        out_r = out.rearrange("(m p) q -> p m q", p=128)
        for mo, ps in ((0, stC), (1, stD)):
            for c in range(4):
                nc.tensor.matmul(ps[:, :NQ], lwt[:, c, ts(mo, 128)],
                                 attn2[:, c, :], start=(c == 0), stop=(c == 3))
            nc.vector.tensor_scalar_add(outsb[:, mo, :], ps[:, :NQ],
                                        bt[:, 4 * mo + 3:4 * mo + 4])
            nc.sync.dma_start(out_r[:, mo], outsb[:, mo, :])

    nc.compile()
    return nc


def _get_nc():
    if "nc" not in _CACHE:
        _CACHE["nc"] = _build_nc()
    return _CACHE["nc"]


def _prep_in_maps(input_x, pe_Q, pe_K, WQ, WK, WV, Q_bias, K_bias, V_bias,
                  lin_w, lin_b):
    bf = ml_dtypes.bfloat16
    x_kT = np.ascontiguousarray(
        np.concatenate([input_x, pe_K], axis=1).T.astype(bf))       # [1024, 4096]
    x_q = np.concatenate([input_x, pe_Q], axis=1)                   # [4096, 1024]
    wq2 = np.ascontiguousarray(
        WQ.transpose(1, 0, 2).reshape(QKD, HID).astype(bf))         # [d,(h,hd)]
    wk2 = np.ascontiguousarray(WK.transpose(1, 0, 2).reshape(QKD, HID).astype(bf))
    wv2 = np.ascontiguousarray(WV.transpose(1, 0, 2).reshape(IND, HID).astype(bf))
    # permuted lin_w.T for the PSUM partition layout: chunk c = 2*mg + b,
    # partition p<32 -> head (4mg+2b) row p; 64<=p<96 -> head (4mg+2b+1)
    # row p-64; other partitions (Z rows + junk) get zero weights.
    lwT = lin_w.T  # [HID_in (h, hd), HID_out]
    lwP = np.zeros((4 * 128, HID), np.float32)
    for c in range(4):
        mg, b = divmod(c, 2)
        h_lo = 4 * mg + 2 * b
        lwP[c * 128 + 0:c * 128 + 32] = lwT[32 * h_lo:32 * h_lo + 32]
        lwP[c * 128 + 64:c * 128 + 96] = lwT[32 * (h_lo + 1):32 * (h_lo + 1) + 32]
    lwPn = np.ascontiguousarray(lwP.astype(bf))
    bias4 = np.zeros((128, 8), np.float32)
    for m in range(2):
        for i, vec in enumerate([Q_bias.reshape(HID), K_bias.reshape(HID),
                                 V_bias.reshape(HID), lin_b.reshape(HID)]):
            bias4[:, 4 * m + i] = vec[128 * m:128 * (m + 1)]
    in_maps = []
    for i in range(NCORES):
        xqT_i = np.ascontiguousarray(
            x_q[i * NQ:(i + 1) * NQ].T.astype(bf))                  # [1024, 512]
        in_maps.append({
            "xkT": x_kT, "xqT": xqT_i, "wq": wq2, "wk": wk2, "wv": wv2,
            "lwP": lwPn, "bias4": bias4,
        })
    return in_maps


def _ensure_ntff_hook():
    """The agent image's antenv lacks axon_hooks; synthesize it from the
    boot script's ctypes NTFF implementation so trace=True works."""
    import types
    try:
        from antenv.axon_hooks import get_axon_ntff_profile_hook  # noqa: F401
        return
    except ImportError:
        pass
    sys.path.insert(0, "/root/.axon_site/trn_agent_boot")
    import trn_boot
    hook = trn_boot._ntff_profile_via_ctypes(
        os.environ.get("PJRT_LIBRARY_PATH", "/opt/axon/libaxon_pjrt.so"))
    mod = types.ModuleType("antenv.axon_hooks")
    mod._hook = hook
    mod.get_axon_ntff_profile_hook = lambda: mod._hook
    mod.set_axon_ntff_profile_hook = lambda h: setattr(mod, "_hook", h)
    sys.modules["antenv.axon_hooks"] = mod


def _run(in_maps, trace=False):
    from concourse.bass_utils import run_bass_kernel_spmd
    if trace:
        _ensure_ntff_hook()
    nc = _get_nc()
    res = run_bass_kernel_spmd(nc, in_maps, core_ids=list(range(NCORES)),
                               trace=trace)
    return res


def kernel(input_x, pe_Q, pe_K, A, WQ, WK, WV, Q_bias, K_bias, V_bias,
           lin_w, lin_b):
    in_maps = _prep_in_maps(
        np.asarray(input_x, np.float32), np.asarray(pe_Q, np.float32),
        np.asarray(pe_K, np.float32), np.asarray(WQ, np.float32),
        np.asarray(WK, np.float32), np.asarray(WV, np.float32),
        np.asarray(Q_bias, np.float32), np.asarray(K_bias, np.float32),
        np.asarray(V_bias, np.float32), np.asarray(lin_w, np.float32),
        np.asarray(lin_b, np.float32))
    res = _run(in_maps)
    out_full = np.empty((N, HID), np.float32)
    for i in range(NCORES):
        out_full[i * NQ:(i + 1) * NQ] = res.results[i]["out"].T
    return out_full


def hw_exec_ns(input_x, pe_Q, pe_K, A, WQ, WK, WV, Q_bias, K_bias, V_bias,
               lin_w, lin_b):
    """Run once with NTFF tracing; returns (exec_time_ns, results)."""
    in_maps = _prep_in_maps(
        np.asarray(input_x, np.float32), np.asarray(pe_Q, np.float32),
        np.asarray(pe_K, np.float32), np.asarray(WQ, np.float32),
        np.asarray(WK, np.float32), np.asarray(WV, np.float32),
        np.asarray(Q_bias, np.float32), np.asarray(K_bias, np.float32),
        np.asarray(V_bias, np.float32), np.asarray(lin_w, np.float32),
        np.asarray(lin_b, np.float32))
    res = _run(in_maps, trace=True)
    return res.exec_time_ns, res


# revision 37
# speedup vs baseline: 1.0298x; 1.0050x over previous
"""Trainium2 Bass kernel for nn_Attention_layer (GNN message passing attention).

Math (see harness reference):
  x_Q = [input_x, pe_Q]  (N, 1024);  x_K = [input_x, pe_K]
  Q = x_Q @ WQ[h] + qb;  K = x_K @ WK[h] + kb;  V = input_x @ WV[h] (+vb=0)
  attn = softmax(Q K^T / 16, axis=k);  out = concat_h(attn @ V) @ lin_w.T + lin_b

Distribution: 8 NeuronCores, query-dim (N) sharded 512 rows/core; K/V work
replicated (no collectives).  Per core, transposed domain (scores^T [k, q]).

Rev B design (head-group-major + PSUM PV accumulation + split exp engines):
  - iteration = (mg, kc): head-group mg in {0,1} (4 heads), k-chunk kc in
    0..31 (128 nodes). All 32 kc of mg=0 first, then mg=1.
  - scores: one 4-bank PSUM tensor st [128, 2048]; 4 matmuls (contraction
    hd=32, row-tiled via tile_position (32j, 0)).
  - exp split across engines by column (rows of a softmax stay on one
    engine so approximation error cancels in Z): ScalarE does true exp on
    cols [0, SPLIT); VectorE does a Schraudolph bit-trick exp on the rest
    (one tensor_scalar affine into int16 bits == bf16 exp, ~1.8% rms,
    zero-mean; confined to 2 of 8 heads -> ~8e-3 output rel err).
  - PV+Z fused: V augmented with a ones column per head (vt [.., 8, 33]);
    one M=33 matmul per head (tile cols 0 / 64) accumulates attn_x^T AND
    the softmax denominator Z directly in a persistent 2-bank PSUM tile
    across all 32 kc (no vector adds, no separate Z matmuls).
  - projections (Q/K/V on PE, bias-add/copy on VectorE; GpSimd cannot
    read PSUM) are interleaved
    just-in-time into the iteration stream, K proj for head-group mg
    scheduled inside mg's own half.
  - epilogue per mg (overlapped with the other half's stream): gather Z
    rows via selector matmul, fast reciprocal, broadcast via outer-product
    matmul, normalize; final linear uses a host-permuted lin_w so the
    PSUM partition layout feeds it directly; lin_w rows are zeroed for
    junk partitions.
"""

import os
import sys
import math
import numpy as np
import ml_dtypes

for _p in ("/opt/trn_rl_repo", "/root/.axon_site/_ro/trn_rl_repo"):
    if os.path.isdir(_p) and _p not in sys.path:
        sys.path.insert(0, _p)

N = 4096
IND = 256          # input_x dim
QKD = 1024         # concat dim for Q/K projections
H = 8              # heads
HD = 32            # head dim
HID = 256          # H * HD
NCORES = 8
NQ = N // NCORES   # 512 query rows per core
SCALE = 1.0 / 16.0  # 1/sqrt(HID)

# exp engine split: ScalarE (true exp) takes score cols [0, SPLIT) = heads
# j0..j2 (tensors stAB+stC); VectorE (Schraudolph) takes head j3 (stD).
SPLIT = 1536
EXP_A = 8.0 / math.log(2.0)        # 128/(16 ln2): bf16-bits slope on raw scores
EXP_B = 16256.0 - 7.4              # 127<<7 minus rms-centering constant

_CACHE = {}


def _build_nc():
    from contextlib import ExitStack
    import concourse.bacc as bacc
    import concourse.tile as tile
    import concourse.mybir as mybir
    from concourse.bass import ds, ts

    f32 = mybir.dt.float32
    bf16 = mybir.dt.bfloat16
    i16 = mybir.dt.int16
    Exp = mybir.ActivationFunctionType.Exp
    mult = mybir.AluOpType.mult
    add = mybir.AluOpType.add

    nc = bacc.Bacc("TRN2", target_bir_lowering=False, debug=False,
                   num_devices=NCORES)

    # Z gather / broadcast selector constants
    selz_np = np.zeros((128, 2), dtype=np.float16)
    selz_np[32, 0] = 1.0
    selz_np[96, 1] = 1.0
    bselm_np = np.zeros((2, 128), dtype=np.float16)
    bselm_np[0, 0:32] = 1.0
    bselm_np[1, 64:96] = 1.0

    # ---- DRAM I/O (per-core shards prepared on host) ----
    xkT = nc.dram_tensor("xkT", [QKD, N], bf16, kind="ExternalInput")   # [x;peK]^T
    wqkx = nc.dram_tensor("wqkx", [QKD, 2 * HID + NQ], bf16,
                          kind="ExternalInput")  # [WQ | WK | x_Q^T blk]
    wv = nc.dram_tensor("wv", [IND, HID], bf16, kind="ExternalInput")
    lwP = nc.dram_tensor("lwP", [4 * 128, HID], bf16, kind="ExternalInput")  # permuted lin_w.T
    bias4 = nc.dram_tensor("bias4", [128, 8], f32, kind="ExternalInput")  # [p, 4m+i]
    out = nc.dram_tensor("out", [HID, NQ], f32, kind="ExternalOutput")   # out^T


    with tile.TileContext(nc) as tc, ExitStack() as ctx:
        consts = ctx.enter_context(tc.tile_pool(name="consts", bufs=1))
        big = ctx.enter_context(tc.tile_pool(name="big", bufs=1))
        ptp = ctx.enter_context(tc.tile_pool(name="ptp", bufs=6))
        stp = ctx.enter_context(tc.tile_pool(name="stp", bufs=1, space="PSUM"))

        # ---- SBUF tiles ----
        xkt = big.tile([128, 8, N], bf16, tag="xkt")       # x_K^T  (8 c-chunks)
        wqkxt = big.tile([128, 8, 2 * HID + NQ], bf16, tag="wqkxt")
        wqt = wqkxt[:, :, ds(0, HID)]                      # [128, 8, 256]
        wkt = wqkxt[:, :, ds(HID, HID)]
        xqt = wqkxt[:, :, ds(2 * HID, NQ)]
        wvt = consts.tile([128, 2, HID], bf16, tag="wvt")
        lwt = consts.tile([128, 4, HID], bf16, tag="lwt")  # permuted lin_w.T
        bt = consts.tile([128, 8], f32, tag="bt")          # [p, 4m+i]
        selz = consts.tile([128, 2], bf16, tag="selz")
        bselm = consts.tile([2, 128], bf16, tag="bselm")

        kt = big.tile([128, 2, N], bf16, tag="kt")         # K^T rows (h,hd)
        qt = big.tile([128, 2, NQ], bf16, tag="qt")        # Q^T
        vt = big.tile([128, 32, 8, 33], bf16, tag="vt")    # V node-major, +ones col
        pvs = big.tile([128, 2, 2, 512], f32, tag="pvs")   # PV+Z psum copies per mg
        attn2 = big.tile([128, 4, 512], bf16, tag="attn2")  # normalized attn_x^T
        zrm = big.tile([2, 2, 2, NQ], f32, tag="zrm")      # 1/Z [row, mg, b, q]
        zrh = big.tile([2, 2, 2, NQ], bf16, tag="zrh")     # bf16 1/Z for PE
        zsb = big.tile([128, 2, NQ], bf16, tag="zsb")      # bf16 copy of pvz
        outsb = big.tile([128, 2, NQ], f32, tag="outsb")

        # ---- persistent PSUM: separate score tensors per reader engine so
        # WAR chains stay independent (tile dep tracking is per-tensor) ----
        stAB = stp.tile([128, 2, NQ], f32, tag="stAB", name="stAB")  # exp1 (ACT)
        stC = stp.tile([128, NQ], f32, tag="stC", name="stC")        # exp2 (ACT)
        stD = stp.tile([128, NQ], f32, tag="stD", name="stD")        # schr (DVE)
        pvz = stp.tile([128, 2, NQ], f32, tag="pvz", name="pvz")     # 2 banks

        # ---- const / weight DMAs, ordered by first consumer ----
        xkT_r = xkT.rearrange("(c p) (n q) -> n p c q", p=128, q=512)
        wqkx_r = wqkx.rearrange("(c p) o -> p c o", p=128)
        nc.sync.dma_start(wqkxt[:], wqkx_r[:])
        nc.sync.dma_start(xkt[:, :, ds(0, 128)], xkT_r[0][:, :, ds(0, 128)])
        nc.sync.dma_start(bt[:], bias4[:])
        nc.sync.dma_start(wvt[:], wv.rearrange("(c p) o -> p c o", p=128))
        nc.sync.dma_start(xkt[:, :, ds(128, 384)], xkT_r[0][:, :, ds(128, 384)])
        for n in range(1, 8):
            nc.sync.dma_start(xkt[:, :, ts(n, 512)], xkT_r[n])
        nc.sync.dma_start(lwt[:], lwP.rearrange("(c p) o -> p c o", p=128))
        nc.sync.dma_start(selz[:], nc.inline_tensor(
            selz_np.astype(ml_dtypes.bfloat16), name="selz_c")[:])
        nc.sync.dma_start(bselm[:], nc.inline_tensor(
            bselm_np.astype(ml_dtypes.bfloat16), name="bselm_c")[:])

        # PE clock warmup: dependency-free dummy matmuls on a locally
        # memset tile start right after the preamble and ramp the PE p-state
        # while the input DMAs land.
        warm = big.tile([128, 512], bf16, tag="warm")
        nc.vector.memset(warm[:], 1.0)
        for w in range(8):
            nc.tensor.matmul(stAB[:, 0, :], warm[:, ds(0, 128)], warm[:],
                             start=True, stop=True)
        # preload the ACT exp table set while DMAs land
        actwarm = consts.tile([8, 16], f32, tag="actwarm")
        nc.vector.memset(actwarm[:], 0.0)
        nc.scalar.activation(actwarm[:], actwarm[:], Exp)
        # ones-column of augmented V (never overwritten: V copies skip col 32)
        for h in range(H):
            nc.vector.memset(vt[:, :, h, ds(32, 1)], 1.0)

        # PV+Z accumulates with start=False onto explicitly zeroed banks
        # (two start=True matmuls sharing a bank would re-zero each other)
        nc.vector.memset(pvz[:], 0.0)

        # ---- projection helpers (PE matmuls + GpSimd bias-add/copy) ----
        def q_proj_unit(m, lo, nmm):
            ps = stp.tile([128, NQ], f32, tag="pzv", bufs=1, name=f"qp{m}_{lo}")
            for c in range(lo, lo + nmm):
                nc.tensor.matmul(ps[:, :NQ], wqt[:, c, ts(m, 128)], xqt[:, c, :],
                                 start=(c == 0), stop=(c == 7))
            if lo + nmm == 8:
                nc.vector.tensor_scalar_add(qt[:, m, :], ps[:, :NQ],
                                            bt[:, 4 * m + 0:4 * m + 1])
            return ps

        qproj_open = {}

        def q_proj(m, half):
            if half == 0:
                qproj_open[m] = q_proj_unit(m, 0, 4)
            else:
                ps = qproj_open.pop(m)
                for c in range(4, 8):
                    nc.tensor.matmul(ps[:, :NQ], wqt[:, c, ts(m, 128)],
                                     xqt[:, c, :], start=False, stop=(c == 7))
                nc.vector.tensor_scalar_add(qt[:, m, :], ps[:, :NQ],
                                            bt[:, 4 * m + 0:4 * m + 1])

        def k_proj_narrow(n, m, lo, w):
            ps = stp.tile([128, NQ], f32, tag="pzk", bufs=1, name=f"kn{n}_{m}_{lo}")
            for c in range(8):
                nc.tensor.matmul(ps[:, :w], wkt[:, c, ts(m, 128)],
                                 xkt[:, c, ds(512 * n + lo, w)],
                                 start=(c == 0), stop=(c == 7))
            nc.vector.tensor_scalar_add(kt[:, m, ds(512 * n + lo, w)], ps[:, :w],
                                        bt[:, 4 * m + 1:4 * m + 2])

        kproj_open = {}

        def k_proj_quarter(n, m, qtr):
            if qtr == 0:
                ps = stp.tile([128, NQ], f32, tag="pzk", bufs=1, name=f"kp{n}_{m}")
                kproj_open[(n, m)] = ps
            else:
                ps = kproj_open[(n, m)]
            for c in range(2 * qtr, 2 * qtr + 2):
                nc.tensor.matmul(ps[:, :512], wkt[:, c, ts(m, 128)],
                                 xkt[:, c, ts(n, 512)],
                                 start=(c == 0), stop=(c == 7))
            if qtr == 3:
                del kproj_open[(n, m)]
                nc.vector.tensor_scalar_add(kt[:, m, ts(n, 512)], ps[:, :512],
                                            bt[:, 4 * m + 1:4 * m + 2])

        def v_proj_unit(kc):
            ps = stp.tile([128, NQ], f32, tag="pzv", bufs=1, name=f"vp{kc}")
            for c in range(2):
                nc.tensor.matmul(ps[:, :HID], xkt[:, c, ds(128 * kc, 128)],
                                 wvt[:, c, :], start=(c == 0), stop=(c == 1))
            # strided copy into augmented V layout (skips the ones column)
            nc.vector.tensor_copy(out=vt[:, kc, :, ds(0, 32)], in_=ps[:, :HID])

        # ---- epilogue per head-group ----
        def epilogue_zsb(b):
            nc.vector.tensor_copy(out=zsb[:, b, :], in_=pvz[:, b])

        def epilogue_copy(mg, b):
            nc.vector.tensor_copy(out=pvs[:, mg, b], in_=pvz[:, b])

        def pvz_clear():
            nc.vector.memset(pvz[:], 0.0)

        def epilogue_zq(mg, b):
            zq = stp.tile([128, NQ], f32, tag="pzv", bufs=1, name=f"zq{mg}_{b}")
            nc.tensor.matmul(zq[ds(0, 2), :NQ], selz[:], zsb[:, b, :],
                             start=True, stop=True)
            nc.vector.reciprocal_approx_fast(zrm[ds(0, 2), mg, b, :],
                                             zq[ds(0, 2), :NQ])
            nc.vector.tensor_copy(out=zrh[ds(0, 2), mg, b, :],
                                  in_=zrm[ds(0, 2), mg, b, :])

        def epilogue_norm(mg, b):
            psb = stp.tile([128, NQ], f32, tag="pzv", bufs=1, name=f"psb{mg}_{b}")
            nc.tensor.matmul(psb[:, :NQ], bselm[:], zrh[:, mg, b, :],
                             start=True, stop=True)
            nc.vector.tensor_tensor(attn2[:, 2 * mg + b, :], pvs[:, mg, b, :],
                                    psb[:, :NQ], mult)

        # ---- minimal prologue: what iteration 0 needs ----
        q_proj(0, 0)
        q_proj(0, 1)
        k_proj_narrow(0, 0, 0, 128)
        v_proj_unit(0)
        k_proj_narrow(0, 0, 128, 384)

        # ---- interleaved work schedule: slot i = iteration i ----
        pre_work = {}
        post_work = {}

        def at(i, fn):
            pre_work.setdefault(i, []).append(fn)

        def at_post(i, fn):
            post_work.setdefault(i, []).append(fn)

        # K proj m=0 tiles 1..7 during the preceding 4 iterations
        for n in range(1, 8):
            for qtr in range(4):
                at(max(1, 4 * (n - 1) + qtr),
                   lambda n=n, qtr=qtr: k_proj_quarter(n, 0, qtr))
        # K proj m=1 tile 0 late in the first half; tiles 1..7 in second half
        at(28, lambda: k_proj_narrow(0, 1, 0, 256))
        at(30, lambda: k_proj_narrow(0, 1, 256, 256))
        for n in range(1, 8):
            for qtr in range(4):
                at(32 + 4 * (n - 1) + qtr,
                   lambda n=n, qtr=qtr: k_proj_quarter(n, 1, qtr))
        # V chunks a couple of iterations ahead of their PV use
        for kc in range(1, 32):
            at(max(0, kc - 2), lambda kc=kc: v_proj_unit(kc))
        # Q proj m=1 before the second half
        at(22, lambda: q_proj(1, 0))
        at(26, lambda: q_proj(1, 1))
        # head-group 0 epilogue, overlapped with the mg=1 stream
        at_post(33, lambda: epilogue_zsb(0))
        at_post(33, lambda: epilogue_zsb(1))
        at_post(34, lambda: epilogue_copy(0, 0))
        at_post(34, lambda: epilogue_zq(0, 0))
        at_post(35, lambda: epilogue_copy(0, 1))
        at_post(35, lambda: epilogue_zq(0, 1))
        at_post(35, lambda: pvz_clear())
        at_post(36, lambda: epilogue_norm(0, 0))
        at_post(37, lambda: epilogue_norm(0, 1))

        # ---- PV+Z unit: matmuls accumulating in persistent PSUM ----
        def pvz_unit(ptA, ptB, mg, kc):
            for b in range(2):
                for half in range(2):
                    j = 2 * b + half
                    h = 4 * mg + j
                    rhs = (ptA[:, j, :] if j < 3 else ptB[:].bitcast(bf16))
                    nc.tensor.matmul(
                        pvz[ds(64 * half, 33), b, :],
                        vt[:, kc, h, :], rhs,
                        start=False, stop=(kc == 31),
                        tile_position=(0, 64 * half),
                        skip_group_check=True)

        # ---- main loop: 64 iterations of (mg, kc) ----
        pending = []
        for i in range(64):
            mg, kc = divmod(i, 32)[0], i % 32
            for fn in pre_work.get(i, []):
                fn()
            ptA = ptp.tile([128, 3, NQ], bf16, tag="ptA", name="ptA")
            ptB = ptp.tile([128, NQ], i16, tag="ptB", name="ptB")
            sdst = [stAB[:, 0, :], stAB[:, 1, :], stC[:, :], stD[:, :]]
            for j in range(4):
                nc.tensor.matmul(
                    sdst[j],
                    kt[ds(32 * j, 32), mg, ds(128 * kc, 128)],
                    qt[ds(32 * j, 32), mg, :],
                    start=True, stop=True,
                    tile_position=(32 * j, 0))
            nc.scalar.activation(ptA[:, 0:2, :], stAB[:], Exp, scale=SCALE)
            nc.scalar.activation(ptA[:, 2, :], stC[:], Exp, scale=SCALE)
            nc.vector.tensor_scalar(ptB[:], stD[:], EXP_A, EXP_B, mult, add)
            # hold mg=1's first PV units two extra slots so the mg=0
            # epilogue (which must finish reading/clearing pvz first) can
            # spread out; catch back up with double flushes.
            n_flush = {34: 0, 35: 0, 36: 2, 37: 2}.get(i, 1)
            for _ in range(n_flush):
                if len(pending) >= 2:
                    pvz_unit(*pending.pop(0))
            pending.append((ptA, ptB, mg, kc))
            for fn in post_work.get(i, []):
                fn()
        for args in pending:
            pvz_unit(*args)

        # ---- tail: head-group 1 epilogue + final linear (lin reuses the
        # dead score PSUM banks; per-bank chains interleave) ----
        def tdummy(n=1):
            for _ in range(n):
                nc.tensor.matmul(stAB[:, 0, :], warm[:, ds(0, 128)], warm[:],
                                 start=True, stop=True)

        epilogue_zsb(0)
        epilogue_zsb(1)
        epilogue_zq(1, 0)
        epilogue_copy(1, 0)
        epilogue_zq(1, 1)
        tdummy(2)
        epilogue_copy(1, 1)
        epilogue_norm(1, 0)
        tdummy(2)
        epilogue_norm(1, 1)
        tdummy(2)
        out_r = out.rearrange("(m p) q -> p m q", p=128)
        for mo, ps in ((0, stC), (1, stD)):
            for c in range(4):
                nc.tensor.matmul(ps[:, :NQ], lwt[:, c, ts(mo, 128)],
                                 attn2[:, c, :], start=(c == 0), stop=(c == 3))
            nc.vector.tensor_scalar_add(outsb[:, mo, :], ps[:, :NQ],
                                        bt[:, 4 * mo + 3:4 * mo + 4])
            nc.sync.dma_start(out_r[:, mo], outsb[:, mo, :])

    nc.compile()
    return nc


def _get_nc():
    if "nc" not in _CACHE:
        _CACHE["nc"] = _build_nc()
    return _CACHE["nc"]


def _prep_in_maps(input_x, pe_Q, pe_K, WQ, WK, WV, Q_bias, K_bias, V_bias,
                  lin_w, lin_b):
    bf = ml_dtypes.bfloat16
    x_kT = np.ascontiguousarray(
        np.concatenate([input_x, pe_K], axis=1).T.astype(bf))       # [1024, 4096]
    x_q = np.concatenate([input_x, pe_Q], axis=1)                   # [4096, 1024]
    wq2 = np.ascontiguousarray(
        WQ.transpose(1, 0, 2).reshape(QKD, HID).astype(bf))         # [d,(h,hd)]
    wk2 = np.ascontiguousarray(WK.transpose(1, 0, 2).reshape(QKD, HID).astype(bf))
    wv2 = np.ascontiguousarray(WV.transpose(1, 0, 2).reshape(IND, HID).astype(bf))
    # permuted lin_w.T for the PSUM partition layout: chunk c = 2*mg + b,
    # partition p<32 -> head (4mg+2b) row p; 64<=p<96 -> head (4mg+2b+1)
    # row p-64; other partitions (Z rows + junk) get zero weights.
    lwT = lin_w.T  # [HID_in (h, hd), HID_out]
    lwP = np.zeros((4 * 128, HID), np.float32)
    for c in range(4):
        mg, b = divmod(c, 2)
        h_lo = 4 * mg + 2 * b
        lwP[c * 128 + 0:c * 128 + 32] = lwT[32 * h_lo:32 * h_lo + 32]
        lwP[c * 128 + 64:c * 128 + 96] = lwT[32 * (h_lo + 1):32 * (h_lo + 1) + 32]
    lwPn = np.ascontiguousarray(lwP.astype(bf))
    bias4 = np.zeros((128, 8), np.float32)
    for m in range(2):
        for i, vec in enumerate([Q_bias.reshape(HID), K_bias.reshape(HID),
                                 V_bias.reshape(HID), lin_b.reshape(HID)]):
            bias4[:, 4 * m + i] = vec[128 * m:128 * (m + 1)]
    in_maps = []
    for i in range(NCORES):
        xqT_i = x_q[i * NQ:(i + 1) * NQ].T.astype(bf)               # [1024, 512]
        wqkx_i = np.ascontiguousarray(
            np.concatenate([wq2, wk2, xqT_i], axis=1))              # [1024, 1024]
        in_maps.append({
            "xkT": x_kT, "wqkx": wqkx_i, "wv": wv2,
            "lwP": lwPn, "bias4": bias4,
        })
    return in_maps


def _ensure_ntff_hook():
    """The agent image's antenv lacks axon_hooks; synthesize it from the
    boot script's ctypes NTFF implementation so trace=True works."""
    import types
    try:
        from antenv.axon_hooks import get_axon_ntff_profile_hook  # noqa: F401
        return
    except ImportError:
        pass
    sys.path.insert(0, "/root/.axon_site/trn_agent_boot")
    import trn_boot
    hook = trn_boot._ntff_profile_via_ctypes(
        os.environ.get("PJRT_LIBRARY_PATH", "/opt/axon/libaxon_pjrt.so"))
    mod = types.ModuleType("antenv.axon_hooks")
    mod._hook = hook
    mod.get_axon_ntff_profile_hook = lambda: mod._hook
    mod.set_axon_ntff_profile_hook = lambda h: setattr(mod, "_hook", h)
    sys.modules["antenv.axon_hooks"] = mod


def _run(in_maps, trace=False):
    from concourse.bass_utils import run_bass_kernel_spmd
    if trace:
        _ensure_ntff_hook()
    nc = _get_nc()
    res = run_bass_kernel_spmd(nc, in_maps, core_ids=list(range(NCORES)),
                               trace=trace)
    return res


def kernel(input_x, pe_Q, pe_K, A, WQ, WK, WV, Q_bias, K_bias, V_bias,
           lin_w, lin_b):
    in_maps = _prep_in_maps(
        np.asarray(input_x, np.float32), np.asarray(pe_Q, np.float32),
        np.asarray(pe_K, np.float32), np.asarray(WQ, np.float32),
        np.asarray(WK, np.float32), np.asarray(WV, np.float32),
        np.asarray(Q_bias, np.float32), np.asarray(K_bias, np.float32),
        np.asarray(V_bias, np.float32), np.asarray(lin_w, np.float32),
        np.asarray(lin_b, np.float32))
    res = _run(in_maps)
    out_full = np.empty((N, HID), np.float32)
    for i in range(NCORES):
        out_full[i * NQ:(i + 1) * NQ] = res.results[i]["out"].T
    return out_full


def hw_exec_ns(input_x, pe_Q, pe_K, A, WQ, WK, WV, Q_bias, K_bias, V_bias,
               lin_w, lin_b):
    """Run once with NTFF tracing; returns (exec_time_ns, results)."""
    in_maps = _prep_in_maps(
        np.asarray(input_x, np.float32), np.asarray(pe_Q, np.float32),
        np.asarray(pe_K, np.float32), np.asarray(WQ, np.float32),
        np.asarray(WK, np.float32), np.asarray(WV, np.float32),
        np.asarray(Q_bias, np.float32), np.asarray(K_bias, np.float32),
        np.asarray(V_bias, np.float32), np.asarray(lin_w, np.float32),
        np.asarray(lin_b, np.float32))
    res = _run(in_maps, trace=True)
    return res.exec_time_ns, res
